# revision 2
# baseline (speedup 1.0000x reference)
"""Trainium2 Bass kernel for the DLSM GNN message-passing model, v2.

Data-parallel over the batch: each of 8 NeuronCores handles 32 nodes of
nodes1 + 32 of nodes2. The sampling indices (fixed RNG columns x input node
ids x input adjacency tables) are computed host-side; the device performs all
feature gathers, neighbor aggregation, and GC/head matmuls.

Key structure per core:
  - Host dedups the ~67k touched feature rows into a <=32768-row fp16 table
    (content-preserving relayout of `features`), so the batched SWDGE
    dma_gather (int16 indices, <=1024 per instruction, 4 queues) can fetch
    all rows in ~67 instructions instead of ~550 serial indirect DMAs.
  - Gather positions are laid out so hop-2 row j of slot (v,h) lands at
    [partition v%128, col (2*(v//128)+h)*10+j] -> neighbor sums are static
    strided DVE reduces; self rows land node-major [v%128, v//128].
  - Compute is fp16 in / f32 PSUM: per 128-node tile, PE-transpose self and
    the two neighbor sums to feature-major, 3-block GC matmul, ACT sigmoid.
    Hop-0 and the three heads reuse the transposed buffers.
"""
import numpy as np
import sys

sys.path.insert(0, '/opt/trn_rl_repo')

import concourse.bass as bass  # noqa: E402
import concourse.tile as tile  # noqa: E402
from concourse import bacc, mybir  # noqa: E402
from concourse.masks import make_identity  # noqa: E402

# ---- problem constants -----------------------------------------------------
N = 200000
F = 128
B = 256
E = 128
D = 64
NCORES = 8
BL = B // NCORES            # 32 base nodes per side per core
NV = 2 * BL * 50            # 3200 hop-1 nodes per core (v-order: s,b,h1,j1)
NT = NV // 128              # 25 tiles
NSLOT = 50                  # hop-2 slots per partition (2 per tile)
N2 = NV * 20                # 64000 hop-2 gather positions
NSB = 3328                  # self(3200) + base(64) + pad(64) positions
GI = 1024                   # max idxs per dma_gather instruction
# hop-2 positions are split into 3 segments, each with its own compacted
# fp16 table (unique rows <= draws < 32768 so int16 indices always fit).
SEG = (25600, 25600, 12800)          # hop-2 positions per segment
TROWS = (25600, 25600, 12800 + 3264)  # table rows (seg2 also serves self+base)

F16 = mybir.dt.float16
F32 = mybir.dt.float32
I16 = mybir.dt.int16
SIG = mybir.ActivationFunctionType.Sigmoid
COPY = mybir.ActivationFunctionType.Copy

# Sampling columns fixed by jax.random.key(42) inside the reference.
S1_C1_OUT = [10, 56, 8, 17, 28, 26, 9, 20, 22, 35, 15, 4, 14, 21, 6, 53, 27,
             47, 49, 46, 41, 13, 63, 38, 54]
S1_C1_IN = [19, 59, 37, 12, 34, 31, 29, 1, 3, 0, 24, 40, 26, 11, 25, 23, 13,
            27, 43, 6, 57, 35, 58, 51, 9]
S1_C2_OUT = [57, 36, 9, 2, 34, 3, 6, 11, 0, 21]
S1_C2_IN = [33, 13, 21, 0, 54, 16, 46, 24, 30, 43]
S2_C1_OUT = [9, 7, 34, 52, 15, 35, 54, 30, 10, 16, 42, 56, 51, 28, 12, 19,
             24, 49, 2, 38, 43, 32, 48, 1, 39]
S2_C1_IN = [53, 47, 39, 57, 37, 27, 4, 20, 36, 31, 60, 38, 12, 43, 3, 21, 25,
            58, 48, 52, 23, 35, 15, 28, 7]
S2_C2_OUT = [41, 25, 9, 57, 45, 62, 42, 37, 31, 63]
S2_C2_IN = [40, 34, 60, 56, 2, 14, 6, 32, 50, 25]


def _gather_chunks(total):
    """Split `total` positions into <=GI chunks, each a multiple of 128."""
    out = []
    pos = 0
    while pos < total:
        n = min(GI, total - pos)
        out.append((pos, n))
        pos += n
    return out


def build_program():
    nc = bacc.Bacc("TRN2", target_bir_lowering=False, debug=False,
                   num_swdge_queues=4)

    tabs = [nc.dram_tensor(f"tab{i}", [TROWS[i], F], F16,
                           kind="ExternalInput") for i in range(3)]
    # all int16 index tensors concatenated: idx2 (4000 cols) + idxsb (208)
    icat_d = nc.dram_tensor("icat", [128, N2 // 16 + NSB // 16], I16,
                            kind="ExternalInput")
    # all weights concatenated: w1(3*128) w0(3*128) wh(9*128) wd(3*64) cols
    WCOLS = 15 * 128 + 3 * D
    wcat_d = nc.dram_tensor("wcat", [128, WCOLS], F16, kind="ExternalInput")
    out_d = nc.dram_tensor("out", [3, D, 2 * BL], F32, kind="ExternalOutput")

    with tile.TileContext(nc) as tc:
        with (
            tc.tile_pool(name="const", bufs=1) as cp,
            tc.tile_pool(name="pers", bufs=1) as bp,
            tc.tile_pool(name="piece", bufs=3) as gp,
            tc.tile_pool(name="fmaj", bufs=4) as fp,
            tc.tile_pool(name="small", bufs=2) as sp_,
            tc.tile_pool(name="ps_t", bufs=2, space="PSUM") as pa,
            tc.tile_pool(name="ps_mm", bufs=1, space="PSUM") as pm,
        ):
            ident = cp.tile([128, 128], F16)
            make_identity(nc, ident[:])

            wcat = cp.tile([128, WCOLS], F16)
            nc.sync.dma_start(out=wcat[:], in_=wcat_d[:, :])
            w1 = [wcat[:, q * 128:(q + 1) * 128] for q in range(3)]
            w0 = [wcat[:, (3 + q) * 128:(4 + q) * 128] for q in range(3)]
            wh = [[wcat[:, (6 + k * 3 + q) * 128:(7 + k * 3 + q) * 128]
                   for q in range(3)] for k in range(3)]
            wd = [wcat[:, 15 * 128 + k * D:15 * 128 + (k + 1) * D]
                  for k in range(3)]

            icat = cp.tile([128, N2 // 16 + NSB // 16], I16)
            nc.sync.dma_start(out=icat[:], in_=icat_d[:, :])
            idx2 = icat[:, 0:N2 // 16]
            idxsb = icat[:, N2 // 16:]

            # ---- self + base feature gather (node-major [q, v//128, f]) ---
            fs = bp.tile([128, (NSB // 128) * F], F16, tag="fs")
            fs3 = fs[:].rearrange("p (c f) -> p c f", f=F)
            qn = 0
            for pos, n in _gather_chunks(NSB):
                nvalid = min(n, 3264 - pos)
                c0 = pos // 128
                nc.gpsimd.dma_gather(
                    fs3[:, c0:c0 + n // 128, :], tabs[2][:, :],
                    idxsb[:, pos // 16:(pos + n) // 16],
                    n, nvalid, F, queue_num=qn)
                qn = (qn + 1) % 4

            # persistent buffers
            fselfT = bp.tile([128, NV], F16, tag="fselfT")
            h1T = bp.tile([128, NV], F16, tag="h1T")
            msum = bp.tile([128, NSLOT * F], F16, tag="msum")
            msum3 = msum[:].rearrange("p (s f) -> p s f", f=F)

            # ---- front-loaded: self transposes, base transpose, hop-0 -----
            for t in range(NV // 128):
                ps_s = pa.tile([128, 128], F16, tag="ps_s", space="PSUM")
                nc.tensor.matmul(out=ps_s[:], lhsT=fs3[:, t, :], rhs=ident[:],
                                 start=True, stop=True, is_transpose=True)
                nc.scalar.activation(out=fselfT[:, t * 128:(t + 1) * 128],
                                     in_=ps_s[:], func=COPY)
            ps_b = pa.tile([128, 128], F16, tag="ps_s", space="PSUM")
            nc.tensor.matmul(out=ps_b[:], lhsT=fs3[:, 25, :], rhs=ident[:],
                             start=True, stop=True, is_transpose=True)
            fbT = sp_.tile([128, 128], F16, tag="fbT")
            nc.scalar.activation(out=fbT[:], in_=ps_b[:], func=COPY)

            m0 = [sp_.tile([128, 64], F16, tag=f"m0_{h}", name=f"m0_{h}")
                  for h in range(2)]
            mh = [sp_.tile([128, 64], F16, tag=f"mh_{h}", name=f"mh_{h}")
                  for h in range(2)]
            with nc.allow_low_precision("fp16 means, tol 2e-2"):
                for h in range(2):
                    nc.vector.tensor_reduce(
                        out=m0[h][:].rearrange("p (g o) -> p g o", o=1),
                        in_=fselfT[:].rearrange("f (g h j) -> f g h j",
                                                h=2, j=25)[:, :, h, :],
                        axis=mybir.AxisListType.X, op=mybir.AluOpType.add)

            ph0 = pm.tile([128, 2 * BL], F32, tag="ph", space="PSUM")
            nc.tensor.matmul(out=ph0[:], lhsT=w0[0], rhs=fbT[:, 0:2 * BL],
                             start=True, stop=False)
            nc.tensor.matmul(out=ph0[:], lhsT=w0[1], rhs=m0[0][:],
                             start=False, stop=False)
            nc.tensor.matmul(out=ph0[:], lhsT=w0[2], rhs=m0[1][:],
                             start=False, stop=True)
            h0T = sp_.tile([128, 2 * BL], F16, tag="h0T")
            nc.scalar.activation(out=h0T[:], in_=ph0[:], func=SIG)

            # ---- hop-2 pipeline: pieces of 2 tiles (40 cols, 5120 idxs) ---
            pieces = []
            pos = 0
            while pos < N2:
                n = min(5120, N2 - pos)
                pieces.append((pos, n))
                pos += n
            for pos, n in pieces:
                seg = 0 if pos < SEG[0] else (1 if pos < SEG[0] + SEG[1]
                                              else 2)
                ptile = gp.tile([128, (n // 128) * F], F16, tag="piece")
                p3 = ptile[:].rearrange("p (c f) -> p c f", f=F)
                for off, gn in _gather_chunks(n):
                    c0 = off // 128
                    nc.gpsimd.dma_gather(
                        p3[:, c0:c0 + gn // 128, :], tabs[seg][:, :],
                        idx2[:, (pos + off) // 16:(pos + off + gn) // 16],
                        gn, gn, F, queue_num=qn)
                    qn = (qn + 1) % 4
                # neighbor sums: contiguous-run add tree (10 -> 5 -> 2+1)
                s0 = (pos // 128) // 10
                ns = (n // 128) // 10
                x4 = ptile[:].rearrange("p (s j f) -> p s j f", j=10, f=F)
                T = gp.tile([128, ns * 5 * F], F16, tag="redT",
                            name=f"redT_{pos}")
                t4 = T[:].rearrange("p (s j f) -> p s j f", j=5, f=F)
                U = gp.tile([128, ns * 2 * F], F16, tag="redU",
                            name=f"redU_{pos}")
                u4 = U[:].rearrange("p (s j f) -> p s j f", j=2, f=F)
                with nc.allow_low_precision("fp16 neighbor sums, tol 2e-2"):
                    nc.vector.tensor_add(out=t4, in0=x4[:, :, 0:5, :],
                                         in1=x4[:, :, 5:10, :])
                    nc.vector.tensor_add(out=u4, in0=t4[:, :, 0:2, :],
                                         in1=t4[:, :, 2:4, :])
                    nc.vector.tensor_add(out=u4[:, :, 0, :],
                                         in0=u4[:, :, 0, :],
                                         in1=u4[:, :, 1, :])
                    nc.vector.tensor_add(out=msum3[:, s0:s0 + ns, :],
                                         in0=u4[:, :, 0, :],
                                         in1=t4[:, :, 4, :])

                # GC for the tiles completed by this piece
                for t in range(s0 // 2, s0 // 2 + ns // 2):
                    ps_o = pa.tile([128, 128], F16, tag="ps_o", space="PSUM")
                    ps_i = pa.tile([128, 128], F16, tag="ps_i", space="PSUM")
                    nc.tensor.matmul(out=ps_o[:], lhsT=msum3[:, 2 * t, :],
                                     rhs=ident[:], start=True, stop=True,
                                     is_transpose=True)
                    nc.tensor.matmul(out=ps_i[:], lhsT=msum3[:, 2 * t + 1, :],
                                     rhs=ident[:], start=True, stop=True,
                                     is_transpose=True)
                    so = fp.tile([128, 128], F16, tag="so")
                    si = fp.tile([128, 128], F16, tag="si")
                    nc.scalar.activation(out=so[:], in_=ps_o[:], func=COPY)
                    nc.scalar.activation(out=si[:], in_=ps_i[:], func=COPY)

                    ph = pm.tile([128, 128], F32, tag="ph", space="PSUM")
                    nc.tensor.matmul(out=ph[:], lhsT=w1[0],
                                     rhs=fselfT[:, t * 128:(t + 1) * 128],
                                     start=True, stop=False)
                    nc.tensor.matmul(out=ph[:], lhsT=w1[1], rhs=so[:],
                                     start=False, stop=False)
                    nc.tensor.matmul(out=ph[:], lhsT=w1[2], rhs=si[:],
                                     start=False, stop=True)
                    nc.scalar.activation(out=h1T[:, t * 128:(t + 1) * 128],
                                         in_=ph[:], func=SIG)

            # ---- layer-1 means + heads (short tail) -----------------------
            with nc.allow_low_precision("fp16 means, tol 2e-2"):
                for h in range(2):
                    nc.vector.tensor_reduce(
                        out=mh[h][:].rearrange("p (g o) -> p g o", o=1),
                        in_=h1T[:].rearrange("f (g h j) -> f g h j",
                                             h=2, j=25)[:, :, h, :],
                        axis=mybir.AxisListType.X, op=mybir.AluOpType.add)

            for k in range(3):
                pz = pm.tile([128, 2 * BL], F32, tag="ph", space="PSUM")
                nc.tensor.matmul(out=pz[:], lhsT=wh[k][0], rhs=h0T[:],
                                 start=True, stop=False)
                nc.tensor.matmul(out=pz[:], lhsT=wh[k][1], rhs=mh[0][:],
                                 start=False, stop=False)
                nc.tensor.matmul(out=pz[:], lhsT=wh[k][2], rhs=mh[1][:],
                                 start=False, stop=True)
                zh = fp.tile([128, 2 * BL], F16, tag="zh")
                nc.scalar.activation(out=zh[:], in_=pz[:], func=SIG)
                po = pm.tile([D, 2 * BL], F32, tag="po", space="PSUM")
                nc.tensor.matmul(out=po[:], lhsT=wd[k], rhs=zh[:],
                                 start=True, stop=True)
                ot = fp.tile([D, 2 * BL], F32, tag="ot")
                nc.vector.tensor_copy(out=ot[:], in_=po[:])
                nc.sync.dma_start(out=out_d[k, :, :], in_=ot[:])

    nc.compile()
    return nc


_NC_CACHE = None


def _get_nc():
    global _NC_CACHE
    if _NC_CACHE is None:
        _NC_CACHE = build_program()
    return _NC_CACHE


def _wrap16(ids, n):
    """Position-ordered ids -> [128, n//16] int16 wrapped, tiled 8x."""
    a = np.asarray(ids, dtype=np.int64)
    assert a.shape[0] == n and n % 16 == 0
    w = a.astype(np.int16).reshape(-1, 16).T  # [16, n//16]
    return np.ascontiguousarray(np.tile(w, (8, 1)))


def host_prep(nodes1, nodes2, neighbors_out, neighbors_in, features,
              W_in, W_mean, W_std, W_pi, Wd_mean, Wd_std, Wd_pi):
    nodes1 = np.asarray(nodes1, dtype=np.int64)
    nodes2 = np.asarray(nodes2, dtype=np.int64)
    nbr_out = np.asarray(neighbors_out, dtype=np.int64)
    nbr_in = np.asarray(neighbors_in, dtype=np.int64)
    feats = np.asarray(features, dtype=np.float32)

    def scale(w, f):
        w = np.array(w, dtype=np.float32, copy=True)
        w[F:] *= np.float32(f)
        return w.astype(np.float16)

    w1 = scale(W_in, 0.1)
    w0 = scale(W_in, 0.04)
    whs = [scale(W_mean, 0.04), scale(W_std, 0.04), scale(W_pi, 0.04)]
    wds = [np.asarray(w, dtype=np.float16) for w in (Wd_mean, Wd_std, Wd_pi)]

    c1o = (np.array(S1_C1_OUT), np.array(S2_C1_OUT))
    c1i = (np.array(S1_C1_IN), np.array(S2_C1_IN))
    c2o = (np.array(S1_C2_OUT), np.array(S2_C2_OUT))
    c2i = (np.array(S1_C2_IN), np.array(S2_C2_IN))

    in_maps = []
    for c in range(NCORES):
        # v-order: s, b, h1, j1  (concat(out25, in25) per base node)
        s1_parts = []
        base = np.concatenate([nodes1[c * BL:(c + 1) * BL],
                               nodes2[c * BL:(c + 1) * BL]])
        for s, nodes in ((0, nodes1), (1, nodes2)):
            bs = nodes[c * BL:(c + 1) * BL]
            no = nbr_out[bs][:, c1o[s]]           # [BL, 25]
            ni = nbr_in[bs][:, c1i[s]]            # [BL, 25]
            s1_parts.append(np.concatenate([no, ni], 1).reshape(-1))
        s1 = np.concatenate(s1_parts)             # [3200] v-order
        # hop-2 ids in (v, h2, j2) order
        h2o = np.concatenate([nbr_out[s1_parts[0]][:, c2o[0]],
                              nbr_out[s1_parts[1]][:, c2o[1]]])
        h2i = np.concatenate([nbr_in[s1_parts[0]][:, c2i[0]],
                              nbr_in[s1_parts[1]][:, c2i[1]]])
        ids2 = np.concatenate([h2o, h2i], 1).reshape(-1)  # [NV*20]

        # gather-position layouts
        v = np.arange(NV)
        R2 = np.empty(N2, dtype=np.int64)
        vv = np.repeat(v, 20)
        h2 = np.tile(np.repeat(np.arange(2), 10), NV)
        j2 = np.tile(np.arange(10), 2 * NV)
        pos2 = (vv % 128) + 128 * (((vv // 128) * 2 + h2) * 10 + j2)
        R2[pos2] = ids2
        R3 = np.full(NSB, -1, dtype=np.int64)
        R3[(v % 128) + 128 * (v // 128)] = s1
        R3[3200 + np.arange(2 * BL)] = base

        # 3 position segments, each with its own compacted table
        segs = [R2[:SEG[0]], R2[SEG[0]:SEG[0] + SEG[1]],
                np.concatenate([R2[SEG[0] + SEG[1]:], R3[:3264]])]
        tables = []
        locs = []
        for i, req in enumerate(segs):
            uniq, inv = np.unique(req, return_inverse=True)
            assert len(uniq) <= TROWS[i]
            table = np.zeros((TROWS[i], F), dtype=np.float16)
            table[:len(uniq)] = feats[uniq].astype(np.float16)
            tables.append(table)
            locs.append(inv)
        L2 = np.concatenate([locs[0], locs[1], locs[2][:SEG[2]]])
        L3 = np.full(NSB, -1, dtype=np.int64)
        L3[:3264] = locs[2][SEG[2]:]

        icat = np.hstack([_wrap16(L2, N2), _wrap16(L3, NSB)])
        blocks = [w1[q * 128:(q + 1) * 128] for q in range(3)]
        blocks += [w0[q * 128:(q + 1) * 128] for q in range(3)]
        for k in range(3):
            blocks += [whs[k][q * 128:(q + 1) * 128] for q in range(3)]
        blocks += wds
        wcat = np.ascontiguousarray(np.hstack(blocks))

        m = {"icat": icat, "wcat": wcat}
        for i in range(3):
            m[f"tab{i}"] = tables[i]
        in_maps.append(m)
    return in_maps


def kernel(nodes1, nodes2, neighbors_out, neighbors_in, features,
           W_in, W_mean, W_std, W_pi, W_ag, W_ad, Wd_mean, Wd_std, Wd_pi,
           _trace=False):
    in_maps = host_prep(nodes1, nodes2, neighbors_out, neighbors_in, features,
                        W_in, W_mean, W_std, W_pi, Wd_mean, Wd_std, Wd_pi)
    nc = _get_nc()
    from concourse.bass_utils import run_bass_kernel_spmd
    res = run_bass_kernel_spmd(nc, in_maps, list(range(NCORES)),
                               trace=_trace)
    if _trace:
        kernel.last_results = res

    out = np.zeros((6, B, D), dtype=np.float32)
    for c in range(NCORES):
        o = res.results[c]["out"]  # [3, D, 64] cols g = s*32+b
        for k in range(3):
            for s in range(2):
                out[s * 3 + k, c * BL:(c + 1) * BL, :] = \
                    o[k][:, s * BL:(s + 1) * BL].T
    return out


# revision 3
# speedup vs baseline: 1.0021x; 1.0021x over previous
"""Trainium2 Bass kernel for the DLSM GNN message-passing model, v2.

Data-parallel over the batch: each of 8 NeuronCores handles 32 nodes of
nodes1 + 32 of nodes2. The sampling indices (fixed RNG columns x input node
ids x input adjacency tables) are computed host-side; the device performs all
feature gathers, neighbor aggregation, and GC/head matmuls.

Key structure per core:
  - Host dedups the ~67k touched feature rows into a <=32768-row fp16 table
    (content-preserving relayout of `features`), so the batched SWDGE
    dma_gather (int16 indices, <=1024 per instruction, 4 queues) can fetch
    all rows in ~67 instructions instead of ~550 serial indirect DMAs.
  - Gather positions are laid out so hop-2 row j of slot (v,h) lands at
    [partition v%128, col (2*(v//128)+h)*10+j] -> neighbor sums are static
    strided DVE reduces; self rows land node-major [v%128, v//128].
  - Compute is fp16 in / f32 PSUM: per 128-node tile, PE-transpose self and
    the two neighbor sums to feature-major, 3-block GC matmul, ACT sigmoid.
    Hop-0 and the three heads reuse the transposed buffers.
"""
import numpy as np
import sys

sys.path.insert(0, '/opt/trn_rl_repo')

import concourse.bass as bass  # noqa: E402
import concourse.tile as tile  # noqa: E402
from concourse import bacc, mybir  # noqa: E402
from concourse.masks import make_identity  # noqa: E402

# ---- problem constants -----------------------------------------------------
N = 200000
F = 128
B = 256
E = 128
D = 64
NCORES = 8
BL = B // NCORES            # 32 base nodes per side per core
NV = 2 * BL * 50            # 3200 hop-1 nodes per core (v-order: s,b,h1,j1)
NT = NV // 128              # 25 tiles
NSLOT = 50                  # hop-2 slots per partition (2 per tile)
N2 = NV * 20                # 64000 hop-2 gather positions
NSB = 3328                  # self(3200) + base(64) + pad(64) positions
GI = 1024                   # max idxs per dma_gather instruction
# hop-2 positions are split into 3 segments, each with its own compacted
# fp16 table (unique rows <= draws < 32768 so int16 indices always fit).
SEG = (25600, 25600, 12800)          # hop-2 positions per segment
TROWS = (25600, 25600, 12800 + 3264)  # table rows (seg2 also serves self+base)

F16 = mybir.dt.float16
F32 = mybir.dt.float32
I16 = mybir.dt.int16
SIG = mybir.ActivationFunctionType.Sigmoid
COPY = mybir.ActivationFunctionType.Copy

# Sampling columns fixed by jax.random.key(42) inside the reference.
S1_C1_OUT = [10, 56, 8, 17, 28, 26, 9, 20, 22, 35, 15, 4, 14, 21, 6, 53, 27,
             47, 49, 46, 41, 13, 63, 38, 54]
S1_C1_IN = [19, 59, 37, 12, 34, 31, 29, 1, 3, 0, 24, 40, 26, 11, 25, 23, 13,
            27, 43, 6, 57, 35, 58, 51, 9]
S1_C2_OUT = [57, 36, 9, 2, 34, 3, 6, 11, 0, 21]
S1_C2_IN = [33, 13, 21, 0, 54, 16, 46, 24, 30, 43]
S2_C1_OUT = [9, 7, 34, 52, 15, 35, 54, 30, 10, 16, 42, 56, 51, 28, 12, 19,
             24, 49, 2, 38, 43, 32, 48, 1, 39]
S2_C1_IN = [53, 47, 39, 57, 37, 27, 4, 20, 36, 31, 60, 38, 12, 43, 3, 21, 25,
            58, 48, 52, 23, 35, 15, 28, 7]
S2_C2_OUT = [41, 25, 9, 57, 45, 62, 42, 37, 31, 63]
S2_C2_IN = [40, 34, 60, 56, 2, 14, 6, 32, 50, 25]


def _gather_chunks(total):
    """Split `total` positions into <=GI chunks, each a multiple of 128."""
    out = []
    pos = 0
    while pos < total:
        n = min(GI, total - pos)
        out.append((pos, n))
        pos += n
    return out


def build_program():
    nc = bacc.Bacc("TRN2", target_bir_lowering=False, debug=False,
                   num_swdge_queues=4)

    tabs = [nc.dram_tensor(f"tab{i}", [TROWS[i], F], F16,
                           kind="ExternalInput") for i in range(3)]
    # all int16 index tensors concatenated: idx2 (4000 cols) + idxsb (208)
    icat_d = nc.dram_tensor("icat", [128, N2 // 16 + NSB // 16], I16,
                            kind="ExternalInput")
    # all weights concatenated: w1(3*128) w0(3*128) wh(9*128) wd(3*64) cols
    WCOLS = 15 * 128 + 3 * D
    wcat_d = nc.dram_tensor("wcat", [128, WCOLS], F16, kind="ExternalInput")
    out_d = nc.dram_tensor("out", [3, D, 2 * BL], F32, kind="ExternalOutput")

    with tile.TileContext(nc) as tc:
        with (
            tc.tile_pool(name="const", bufs=1) as cp,
            tc.tile_pool(name="pers", bufs=1) as bp,
            tc.tile_pool(name="piece", bufs=3) as gp,
            tc.tile_pool(name="fmaj", bufs=4) as fp,
            tc.tile_pool(name="small", bufs=2) as sp_,
            tc.tile_pool(name="ps_t", bufs=2, space="PSUM") as pa,
            tc.tile_pool(name="ps_mm", bufs=1, space="PSUM") as pm,
        ):
            ident = cp.tile([128, 128], F16)
            make_identity(nc, ident[:])

            wcat = cp.tile([128, WCOLS], F16)
            nc.sync.dma_start(out=wcat[:], in_=wcat_d[:, :])
            w1 = [wcat[:, q * 128:(q + 1) * 128] for q in range(3)]
            w0 = [wcat[:, (3 + q) * 128:(4 + q) * 128] for q in range(3)]
            wh = [[wcat[:, (6 + k * 3 + q) * 128:(7 + k * 3 + q) * 128]
                   for q in range(3)] for k in range(3)]
            wd = [wcat[:, 15 * 128 + k * D:15 * 128 + (k + 1) * D]
                  for k in range(3)]

            icat = cp.tile([128, N2 // 16 + NSB // 16], I16)
            nc.sync.dma_start(out=icat[:], in_=icat_d[:, :])
            idx2 = icat[:, 0:N2 // 16]
            idxsb = icat[:, N2 // 16:]

            # ---- self + base feature gather (node-major [q, v//128, f]) ---
            fs = bp.tile([128, (NSB // 128) * F], F16, tag="fs")
            fs3 = fs[:].rearrange("p (c f) -> p c f", f=F)
            qn = 0
            for pos, n in _gather_chunks(NSB):
                nvalid = min(n, 3264 - pos)
                c0 = pos // 128
                nc.gpsimd.dma_gather(
                    fs3[:, c0:c0 + n // 128, :], tabs[2][:, :],
                    idxsb[:, pos // 16:(pos + n) // 16],
                    n, nvalid, F, queue_num=qn)
                qn = (qn + 1) % 4

            # persistent buffers
            fselfT = bp.tile([128, NV], F16, tag="fselfT")
            h1T = bp.tile([128, NV], F16, tag="h1T")
            msum = bp.tile([128, NSLOT * F], F16, tag="msum")
            msum3 = msum[:].rearrange("p (s f) -> p s f", f=F)

            # ---- front-loaded: self transposes, base transpose, hop-0 -----
            for t in range(NV // 128):
                ps_s = pa.tile([128, 128], F16, tag="ps_s", space="PSUM")
                nc.tensor.matmul(out=ps_s[:], lhsT=fs3[:, t, :], rhs=ident[:],
                                 start=True, stop=True, is_transpose=True)
                nc.scalar.activation(out=fselfT[:, t * 128:(t + 1) * 128],
                                     in_=ps_s[:], func=COPY)
            ps_b = pa.tile([128, 128], F16, tag="ps_s", space="PSUM")
            nc.tensor.matmul(out=ps_b[:], lhsT=fs3[:, 25, :], rhs=ident[:],
                             start=True, stop=True, is_transpose=True)
            fbT = sp_.tile([128, 128], F16, tag="fbT")
            nc.scalar.activation(out=fbT[:], in_=ps_b[:], func=COPY)

            m0 = [sp_.tile([128, 64], F16, tag=f"m0_{h}", name=f"m0_{h}")
                  for h in range(2)]
            mh = [sp_.tile([128, 64], F16, tag=f"mh_{h}", name=f"mh_{h}")
                  for h in range(2)]
            with nc.allow_low_precision("fp16 means, tol 2e-2"):
                for h in range(2):
                    nc.vector.tensor_reduce(
                        out=m0[h][:].rearrange("p (g o) -> p g o", o=1),
                        in_=fselfT[:].rearrange("f (g h j) -> f g h j",
                                                h=2, j=25)[:, :, h, :],
                        axis=mybir.AxisListType.X, op=mybir.AluOpType.add)

            ph0 = pm.tile([128, 2 * BL], F32, tag="ph", space="PSUM")
            nc.tensor.matmul(out=ph0[:], lhsT=w0[0], rhs=fbT[:, 0:2 * BL],
                             start=True, stop=False)
            nc.tensor.matmul(out=ph0[:], lhsT=w0[1], rhs=m0[0][:],
                             start=False, stop=False)
            nc.tensor.matmul(out=ph0[:], lhsT=w0[2], rhs=m0[1][:],
                             start=False, stop=True)
            h0T = sp_.tile([128, 2 * BL], F16, tag="h0T")
            nc.scalar.activation(out=h0T[:], in_=ph0[:], func=SIG)

            # ---- hop-2 pipeline: pieces of 2 tiles (40 cols, 5120 idxs) ---
            pieces = []
            pos = 0
            while pos < N2:
                n = min(5120, N2 - pos)
                pieces.append((pos, n))
                pos += n
            for pos, n in pieces:
                seg = 0 if pos < SEG[0] else (1 if pos < SEG[0] + SEG[1]
                                              else 2)
                ptile = gp.tile([128, (n // 128) * F], F16, tag="piece")
                p3 = ptile[:].rearrange("p (c f) -> p c f", f=F)
                for off, gn in _gather_chunks(n):
                    c0 = off // 128
                    nc.gpsimd.dma_gather(
                        p3[:, c0:c0 + gn // 128, :], tabs[seg][:, :],
                        idx2[:, (pos + off) // 16:(pos + off + gn) // 16],
                        gn, gn, F, queue_num=qn)
                    qn = (qn + 1) % 4
                # neighbor sums: contiguous-run add tree (10 -> 5 -> 2+1)
                s0 = (pos // 128) // 10
                ns = (n // 128) // 10
                x4 = ptile[:].rearrange("p (s j f) -> p s j f", j=10, f=F)
                T = gp.tile([128, ns * 5 * F], F16, tag="redT",
                            name=f"redT_{pos}")
                t4 = T[:].rearrange("p (s j f) -> p s j f", j=5, f=F)
                U = gp.tile([128, ns * 2 * F], F16, tag="redU",
                            name=f"redU_{pos}")
                u4 = U[:].rearrange("p (s j f) -> p s j f", j=2, f=F)
                with nc.allow_low_precision("fp16 neighbor sums, tol 2e-2"):
                    nc.vector.tensor_add(out=t4, in0=x4[:, :, 0:5, :],
                                         in1=x4[:, :, 5:10, :])
                    nc.vector.tensor_add(out=u4, in0=t4[:, :, 0:2, :],
                                         in1=t4[:, :, 2:4, :])
                    nc.vector.tensor_add(out=u4[:, :, 0, :],
                                         in0=u4[:, :, 0, :],
                                         in1=u4[:, :, 1, :])
                    nc.vector.tensor_add(out=msum3[:, s0:s0 + ns, :],
                                         in0=u4[:, :, 0, :],
                                         in1=t4[:, :, 4, :])

                # GC for the tiles completed by this piece
                for t in range(s0 // 2, s0 // 2 + ns // 2):
                    ps_o = pa.tile([128, 128], F16, tag="ps_o", space="PSUM")
                    ps_i = pa.tile([128, 128], F16, tag="ps_i", space="PSUM")
                    nc.tensor.matmul(out=ps_o[:], lhsT=msum3[:, 2 * t, :],
                                     rhs=ident[:], start=True, stop=True,
                                     is_transpose=True)
                    nc.tensor.matmul(out=ps_i[:], lhsT=msum3[:, 2 * t + 1, :],
                                     rhs=ident[:], start=True, stop=True,
                                     is_transpose=True)
                    so = fp.tile([128, 128], F16, tag="so")
                    si = fp.tile([128, 128], F16, tag="si")
                    nc.scalar.activation(out=so[:], in_=ps_o[:], func=COPY)
                    nc.scalar.activation(out=si[:], in_=ps_i[:], func=COPY)

                    ph = pm.tile([128, 128], F32, tag="ph", space="PSUM")
                    nc.tensor.matmul(out=ph[:], lhsT=w1[0],
                                     rhs=fselfT[:, t * 128:(t + 1) * 128],
                                     start=True, stop=False)
                    nc.tensor.matmul(out=ph[:], lhsT=w1[1], rhs=so[:],
                                     start=False, stop=False)
                    nc.tensor.matmul(out=ph[:], lhsT=w1[2], rhs=si[:],
                                     start=False, stop=True)
                    nc.scalar.activation(out=h1T[:, t * 128:(t + 1) * 128],
                                         in_=ph[:], func=SIG)

            # ---- layer-1 means + heads (short tail) -----------------------
            with nc.allow_low_precision("fp16 means, tol 2e-2"):
                for h in range(2):
                    nc.vector.tensor_reduce(
                        out=mh[h][:].rearrange("p (g o) -> p g o", o=1),
                        in_=h1T[:].rearrange("f (g h j) -> f g h j",
                                             h=2, j=25)[:, :, h, :],
                        axis=mybir.AxisListType.X, op=mybir.AluOpType.add)

            for k in range(3):
                pz = pm.tile([128, 2 * BL], F32, tag="ph", space="PSUM")
                nc.tensor.matmul(out=pz[:], lhsT=wh[k][0], rhs=h0T[:],
                                 start=True, stop=False)
                nc.tensor.matmul(out=pz[:], lhsT=wh[k][1], rhs=mh[0][:],
                                 start=False, stop=False)
                nc.tensor.matmul(out=pz[:], lhsT=wh[k][2], rhs=mh[1][:],
                                 start=False, stop=True)
                zh = fp.tile([128, 2 * BL], F16, tag="zh")
                nc.scalar.activation(out=zh[:], in_=pz[:], func=SIG)
                po = pm.tile([D, 2 * BL], F32, tag="po", space="PSUM")
                nc.tensor.matmul(out=po[:], lhsT=wd[k], rhs=zh[:],
                                 start=True, stop=True)
                ot = fp.tile([D, 2 * BL], F32, tag="ot")
                nc.vector.tensor_copy(out=ot[:], in_=po[:])
                nc.sync.dma_start(out=out_d[k, :, :], in_=ot[:])

    nc.compile()
    return nc


_NC_CACHE = None


def _get_nc():
    global _NC_CACHE
    if _NC_CACHE is None:
        _NC_CACHE = build_program()
    return _NC_CACHE


def _wrap16(ids, n):
    """Position-ordered ids -> [128, n//16] int16 wrapped, tiled 8x."""
    a = np.asarray(ids, dtype=np.int64)
    assert a.shape[0] == n and n % 16 == 0
    w = a.astype(np.int16).reshape(-1, 16).T  # [16, n//16]
    return np.ascontiguousarray(np.tile(w, (8, 1)))


def host_prep(nodes1, nodes2, neighbors_out, neighbors_in, features,
              W_in, W_mean, W_std, W_pi, Wd_mean, Wd_std, Wd_pi):
    nodes1 = np.asarray(nodes1, dtype=np.int64)
    nodes2 = np.asarray(nodes2, dtype=np.int64)
    nbr_out = np.asarray(neighbors_out, dtype=np.int64)
    nbr_in = np.asarray(neighbors_in, dtype=np.int64)
    feats = np.asarray(features, dtype=np.float32)

    def scale(w, f):
        w = np.array(w, dtype=np.float32, copy=True)
        w[F:] *= np.float32(f)
        return w.astype(np.float16)

    w1 = scale(W_in, 0.1)
    w0 = scale(W_in, 0.04)
    whs = [scale(W_mean, 0.04), scale(W_std, 0.04), scale(W_pi, 0.04)]
    wds = [np.asarray(w, dtype=np.float16) for w in (Wd_mean, Wd_std, Wd_pi)]

    c1o = (np.array(S1_C1_OUT), np.array(S2_C1_OUT))
    c1i = (np.array(S1_C1_IN), np.array(S2_C1_IN))
    c2o = (np.array(S1_C2_OUT), np.array(S2_C2_OUT))
    c2i = (np.array(S1_C2_IN), np.array(S2_C2_IN))

    in_maps = []
    for c in range(NCORES):
        # v-order: s, b, h1, j1  (concat(out25, in25) per base node)
        s1_parts = []
        base = np.concatenate([nodes1[c * BL:(c + 1) * BL],
                               nodes2[c * BL:(c + 1) * BL]])
        for s, nodes in ((0, nodes1), (1, nodes2)):
            bs = nodes[c * BL:(c + 1) * BL]
            no = nbr_out[bs][:, c1o[s]]           # [BL, 25]
            ni = nbr_in[bs][:, c1i[s]]            # [BL, 25]
            s1_parts.append(np.concatenate([no, ni], 1).reshape(-1))
        s1 = np.concatenate(s1_parts)             # [3200] v-order
        # hop-2 ids in (v, h2, j2) order
        h2o = np.concatenate([nbr_out[s1_parts[0]][:, c2o[0]],
                              nbr_out[s1_parts[1]][:, c2o[1]]])
        h2i = np.concatenate([nbr_in[s1_parts[0]][:, c2i[0]],
                              nbr_in[s1_parts[1]][:, c2i[1]]])
        ids2 = np.concatenate([h2o, h2i], 1).reshape(-1)  # [NV*20]

        # gather-position layouts
        v = np.arange(NV)
        R2 = np.empty(N2, dtype=np.int64)
        vv = np.repeat(v, 20)
        h2 = np.tile(np.repeat(np.arange(2), 10), NV)
        j2 = np.tile(np.arange(10), 2 * NV)
        pos2 = (vv % 128) + 128 * (((vv // 128) * 2 + h2) * 10 + j2)
        R2[pos2] = ids2
        R3 = np.full(NSB, -1, dtype=np.int64)
        R3[(v % 128) + 128 * (v // 128)] = s1
        R3[3200 + np.arange(2 * BL)] = base

        # 3 position segments, each with its own compacted table
        segs = [R2[:SEG[0]], R2[SEG[0]:SEG[0] + SEG[1]],
                np.concatenate([R2[SEG[0] + SEG[1]:], R3[:3264]])]
        tables = []
        locs = []
        for i, req in enumerate(segs):
            # first-use table order: first occurrences walk the table
            # near-sequentially -> HBM row-buffer friendly gathers
            uniq, first_idx, inv = np.unique(req, return_index=True,
                                             return_inverse=True)
            assert len(uniq) <= TROWS[i]
            order = np.argsort(first_idx)
            rank = np.empty_like(order)
            rank[order] = np.arange(len(order))
            table = np.zeros((TROWS[i], F), dtype=np.float16)
            table[:len(uniq)] = feats[uniq[order]].astype(np.float16)
            tables.append(table)
            locs.append(rank[inv])
        L2 = np.concatenate([locs[0], locs[1], locs[2][:SEG[2]]])
        L3 = np.full(NSB, -1, dtype=np.int64)
        L3[:3264] = locs[2][SEG[2]:]

        icat = np.hstack([_wrap16(L2, N2), _wrap16(L3, NSB)])
        blocks = [w1[q * 128:(q + 1) * 128] for q in range(3)]
        blocks += [w0[q * 128:(q + 1) * 128] for q in range(3)]
        for k in range(3):
            blocks += [whs[k][q * 128:(q + 1) * 128] for q in range(3)]
        blocks += wds
        wcat = np.ascontiguousarray(np.hstack(blocks))

        m = {"icat": icat, "wcat": wcat}
        for i in range(3):
            m[f"tab{i}"] = tables[i]
        in_maps.append(m)
    return in_maps


def kernel(nodes1, nodes2, neighbors_out, neighbors_in, features,
           W_in, W_mean, W_std, W_pi, W_ag, W_ad, Wd_mean, Wd_std, Wd_pi,
           _trace=False):
    in_maps = host_prep(nodes1, nodes2, neighbors_out, neighbors_in, features,
                        W_in, W_mean, W_std, W_pi, Wd_mean, Wd_std, Wd_pi)
    nc = _get_nc()
    from concourse.bass_utils import run_bass_kernel_spmd
    res = run_bass_kernel_spmd(nc, in_maps, list(range(NCORES)),
                               trace=_trace)
    if _trace:
        kernel.last_results = res

    out = np.zeros((6, B, D), dtype=np.float32)
    for c in range(NCORES):
        o = res.results[c]["out"]  # [3, D, 64] cols g = s*32+b
        for k in range(3):
            for s in range(2):
                out[s * 3 + k, c * BL:(c + 1) * BL, :] = \
                    o[k][:, s * BL:(s + 1) * BL].T
    return out


# revision 4
# speedup vs baseline: 1.1725x; 1.1700x over previous
"""Trainium2 Bass kernel for the DLSM GNN message-passing model, v2.

Data-parallel over the batch: each of 8 NeuronCores handles 32 nodes of
nodes1 + 32 of nodes2. The sampling indices (fixed RNG columns x input node
ids x input adjacency tables) are computed host-side; the device performs all
feature gathers, neighbor aggregation, and GC/head matmuls.

Key structure per core:
  - Host dedups the ~67k touched feature rows into a <=32768-row fp16 table
    (content-preserving relayout of `features`), so the batched SWDGE
    dma_gather (int16 indices, <=1024 per instruction, 4 queues) can fetch
    all rows in ~67 instructions instead of ~550 serial indirect DMAs.
  - Gather positions are laid out so hop-2 row j of slot (v,h) lands at
    [partition v%128, col (2*(v//128)+h)*10+j] -> neighbor sums are static
    strided DVE reduces; self rows land node-major [v%128, v//128].
  - Compute is fp16 in / f32 PSUM: per 128-node tile, PE-transpose self and
    the two neighbor sums to feature-major, 3-block GC matmul, ACT sigmoid.
    Hop-0 and the three heads reuse the transposed buffers.
"""
import numpy as np
import sys

sys.path.insert(0, '/opt/trn_rl_repo')

import concourse.bass as bass  # noqa: E402
import concourse.tile as tile  # noqa: E402
from concourse import bacc, mybir  # noqa: E402
from concourse.masks import make_identity  # noqa: E402

# ---- problem constants -----------------------------------------------------
N = 200000
F = 128
B = 256
E = 128
D = 64
NCORES = 8
BL = B // NCORES            # 32 base nodes per side per core
NV = 2 * BL * 50            # 3200 hop-1 nodes per core (v-order: s,b,h1,j1)
NT = NV // 128              # 25 tiles
NSLOT = 50                  # hop-2 slots per partition (2 per tile)
N2 = NV * 20                # 64000 hop-2 gather positions
NSB = 3328                  # self(3200) + base(64) + pad(64) positions
GI = 1024                   # max idxs per dma_gather instruction
import os as _os
SINGLE_PACKET = _os.environ.get('K_SP', '1') == '1'
# hop-2 positions are split into 3 segments, each with its own compacted
# fp16 table (unique rows <= draws < 32768 so int16 indices always fit).
SEG = (25600, 25600, 12800)          # hop-2 positions per segment
TROWS = (25600, 25600, 12800 + 3264)  # table rows (seg2 also serves self+base)

F16 = mybir.dt.float16
F32 = mybir.dt.float32
I16 = mybir.dt.int16
SIG = mybir.ActivationFunctionType.Sigmoid
COPY = mybir.ActivationFunctionType.Copy

# Sampling columns fixed by jax.random.key(42) inside the reference.
S1_C1_OUT = [10, 56, 8, 17, 28, 26, 9, 20, 22, 35, 15, 4, 14, 21, 6, 53, 27,
             47, 49, 46, 41, 13, 63, 38, 54]
S1_C1_IN = [19, 59, 37, 12, 34, 31, 29, 1, 3, 0, 24, 40, 26, 11, 25, 23, 13,
            27, 43, 6, 57, 35, 58, 51, 9]
S1_C2_OUT = [57, 36, 9, 2, 34, 3, 6, 11, 0, 21]
S1_C2_IN = [33, 13, 21, 0, 54, 16, 46, 24, 30, 43]
S2_C1_OUT = [9, 7, 34, 52, 15, 35, 54, 30, 10, 16, 42, 56, 51, 28, 12, 19,
             24, 49, 2, 38, 43, 32, 48, 1, 39]
S2_C1_IN = [53, 47, 39, 57, 37, 27, 4, 20, 36, 31, 60, 38, 12, 43, 3, 21, 25,
            58, 48, 52, 23, 35, 15, 28, 7]
S2_C2_OUT = [41, 25, 9, 57, 45, 62, 42, 37, 31, 63]
S2_C2_IN = [40, 34, 60, 56, 2, 14, 6, 32, 50, 25]


def _gather_chunks(total):
    """Split `total` positions into <=GI chunks, each a multiple of 128."""
    out = []
    pos = 0
    while pos < total:
        n = min(GI, total - pos)
        out.append((pos, n))
        pos += n
    return out


def build_program():
    nc = bacc.Bacc("TRN2", target_bir_lowering=False, debug=False,
                   num_swdge_queues=4)

    tabs = [nc.dram_tensor(f"tab{i}", [TROWS[i], F], F16,
                           kind="ExternalInput") for i in range(3)]
    # all int16 index tensors concatenated: idx2 (4000 cols) + idxsb (208)
    icat_d = nc.dram_tensor("icat", [128, N2 // 16 + NSB // 16], I16,
                            kind="ExternalInput")
    # all weights concatenated: w1(3*128) w0(3*128) wh(9*128) wd(3*64) cols
    WCOLS = 15 * 128 + 3 * D
    wcat_d = nc.dram_tensor("wcat", [128, WCOLS], F16, kind="ExternalInput")
    out_d = nc.dram_tensor("out", [3, D, 2 * BL], F32, kind="ExternalOutput")

    with tile.TileContext(nc) as tc:
        with (
            tc.tile_pool(name="const", bufs=1) as cp,
            tc.tile_pool(name="pers", bufs=1) as bp,
            tc.tile_pool(name="piece", bufs=3) as gp,
            tc.tile_pool(name="fmaj", bufs=4) as fp,
            tc.tile_pool(name="small", bufs=2) as sp_,
            tc.tile_pool(name="ps_t", bufs=2, space="PSUM") as pa,
            tc.tile_pool(name="ps_mm", bufs=1, space="PSUM") as pm,
        ):
            ident = cp.tile([128, 128], F16)
            make_identity(nc, ident[:])

            wcat = cp.tile([128, WCOLS], F16)
            nc.sync.dma_start(out=wcat[:], in_=wcat_d[:, :])
            w1 = [wcat[:, q * 128:(q + 1) * 128] for q in range(3)]
            w0 = [wcat[:, (3 + q) * 128:(4 + q) * 128] for q in range(3)]
            wh = [[wcat[:, (6 + k * 3 + q) * 128:(7 + k * 3 + q) * 128]
                   for q in range(3)] for k in range(3)]
            wd = [wcat[:, 15 * 128 + k * D:15 * 128 + (k + 1) * D]
                  for k in range(3)]

            icat = cp.tile([128, N2 // 16 + NSB // 16], I16)
            nc.sync.dma_start(out=icat[:], in_=icat_d[:, :])
            idx2 = icat[:, 0:N2 // 16]
            idxsb = icat[:, N2 // 16:]

            # ---- self + base feature gather (node-major [q, v//128, f]) ---
            fs = bp.tile([128, (NSB // 128) * F], F16, tag="fs")
            fs3 = fs[:].rearrange("p (c f) -> p c f", f=F)
            qn = 0
            for pos, n in _gather_chunks(NSB):
                nvalid = min(n, 3264 - pos)
                c0 = pos // 128
                nc.gpsimd.dma_gather(
                    fs3[:, c0:c0 + n // 128, :], tabs[2][:, :],
                    idxsb[:, pos // 16:(pos + n) // 16],
                    n, nvalid, F, queue_num=qn,
                    single_packet=SINGLE_PACKET)
                qn = (qn + 1) % 4

            # persistent buffers
            fselfT = bp.tile([128, NV], F16, tag="fselfT")
            h1T = bp.tile([128, NV], F16, tag="h1T")
            msum = bp.tile([128, NSLOT * F], F16, tag="msum")
            msum3 = msum[:].rearrange("p (s f) -> p s f", f=F)

            # ---- front-loaded: self transposes, base transpose, hop-0 -----
            for t in range(NV // 128):
                ps_s = pa.tile([128, 128], F16, tag="ps_s", space="PSUM")
                nc.tensor.matmul(out=ps_s[:], lhsT=fs3[:, t, :], rhs=ident[:],
                                 start=True, stop=True, is_transpose=True)
                nc.scalar.activation(out=fselfT[:, t * 128:(t + 1) * 128],
                                     in_=ps_s[:], func=COPY)
            ps_b = pa.tile([128, 128], F16, tag="ps_s", space="PSUM")
            nc.tensor.matmul(out=ps_b[:], lhsT=fs3[:, 25, :], rhs=ident[:],
                             start=True, stop=True, is_transpose=True)
            fbT = sp_.tile([128, 128], F16, tag="fbT")
            nc.scalar.activation(out=fbT[:], in_=ps_b[:], func=COPY)

            m0 = [sp_.tile([128, 64], F16, tag=f"m0_{h}", name=f"m0_{h}")
                  for h in range(2)]
            mh = [sp_.tile([128, 64], F16, tag=f"mh_{h}", name=f"mh_{h}")
                  for h in range(2)]
            with nc.allow_low_precision("fp16 means, tol 2e-2"):
                for h in range(2):
                    nc.vector.tensor_reduce(
                        out=m0[h][:].rearrange("p (g o) -> p g o", o=1),
                        in_=fselfT[:].rearrange("f (g h j) -> f g h j",
                                                h=2, j=25)[:, :, h, :],
                        axis=mybir.AxisListType.X, op=mybir.AluOpType.add)

            ph0 = pm.tile([128, 2 * BL], F32, tag="ph", space="PSUM")
            nc.tensor.matmul(out=ph0[:], lhsT=w0[0], rhs=fbT[:, 0:2 * BL],
                             start=True, stop=False)
            nc.tensor.matmul(out=ph0[:], lhsT=w0[1], rhs=m0[0][:],
                             start=False, stop=False)
            nc.tensor.matmul(out=ph0[:], lhsT=w0[2], rhs=m0[1][:],
                             start=False, stop=True)
            h0T = sp_.tile([128, 2 * BL], F16, tag="h0T")
            nc.scalar.activation(out=h0T[:], in_=ph0[:], func=SIG)

            # ---- hop-2 pipeline: pieces of 2 tiles (40 cols, 5120 idxs) ---
            pieces = []
            pos = 0
            while pos < N2:
                n = min(5120, N2 - pos)
                pieces.append((pos, n))
                pos += n
            for pos, n in pieces:
                seg = 0 if pos < SEG[0] else (1 if pos < SEG[0] + SEG[1]
                                              else 2)
                ptile = gp.tile([128, (n // 128) * F], F16, tag="piece")
                p3 = ptile[:].rearrange("p (c f) -> p c f", f=F)
                for off, gn in _gather_chunks(n):
                    c0 = off // 128
                    nc.gpsimd.dma_gather(
                        p3[:, c0:c0 + gn // 128, :], tabs[seg][:, :],
                        idx2[:, (pos + off) // 16:(pos + off + gn) // 16],
                        gn, gn, F, queue_num=qn,
                        single_packet=SINGLE_PACKET)
                    qn = (qn + 1) % 4
                # neighbor sums: contiguous-run add tree (10 -> 5 -> 2+1)
                s0 = (pos // 128) // 10
                ns = (n // 128) // 10
                x4 = ptile[:].rearrange("p (s j f) -> p s j f", j=10, f=F)
                T = gp.tile([128, ns * 5 * F], F16, tag="redT",
                            name=f"redT_{pos}")
                t4 = T[:].rearrange("p (s j f) -> p s j f", j=5, f=F)
                U = gp.tile([128, ns * 2 * F], F16, tag="redU",
                            name=f"redU_{pos}")
                u4 = U[:].rearrange("p (s j f) -> p s j f", j=2, f=F)
                with nc.allow_low_precision("fp16 neighbor sums, tol 2e-2"):
                    nc.vector.tensor_add(out=t4, in0=x4[:, :, 0:5, :],
                                         in1=x4[:, :, 5:10, :])
                    nc.vector.tensor_add(out=u4, in0=t4[:, :, 0:2, :],
                                         in1=t4[:, :, 2:4, :])
                    nc.vector.tensor_add(out=u4[:, :, 0, :],
                                         in0=u4[:, :, 0, :],
                                         in1=u4[:, :, 1, :])
                    nc.vector.tensor_add(out=msum3[:, s0:s0 + ns, :],
                                         in0=u4[:, :, 0, :],
                                         in1=t4[:, :, 4, :])

                # GC for the tiles completed by this piece
                for t in range(s0 // 2, s0 // 2 + ns // 2):
                    ps_o = pa.tile([128, 128], F16, tag="ps_o", space="PSUM")
                    ps_i = pa.tile([128, 128], F16, tag="ps_i", space="PSUM")
                    nc.tensor.matmul(out=ps_o[:], lhsT=msum3[:, 2 * t, :],
                                     rhs=ident[:], start=True, stop=True,
                                     is_transpose=True)
                    nc.tensor.matmul(out=ps_i[:], lhsT=msum3[:, 2 * t + 1, :],
                                     rhs=ident[:], start=True, stop=True,
                                     is_transpose=True)
                    so = fp.tile([128, 128], F16, tag="so")
                    si = fp.tile([128, 128], F16, tag="si")
                    nc.scalar.activation(out=so[:], in_=ps_o[:], func=COPY)
                    nc.scalar.activation(out=si[:], in_=ps_i[:], func=COPY)

                    ph = pm.tile([128, 128], F32, tag="ph", space="PSUM")
                    nc.tensor.matmul(out=ph[:], lhsT=w1[0],
                                     rhs=fselfT[:, t * 128:(t + 1) * 128],
                                     start=True, stop=False)
                    nc.tensor.matmul(out=ph[:], lhsT=w1[1], rhs=so[:],
                                     start=False, stop=False)
                    nc.tensor.matmul(out=ph[:], lhsT=w1[2], rhs=si[:],
                                     start=False, stop=True)
                    nc.scalar.activation(out=h1T[:, t * 128:(t + 1) * 128],
                                         in_=ph[:], func=SIG)

            # ---- layer-1 means + heads (short tail) -----------------------
            with nc.allow_low_precision("fp16 means, tol 2e-2"):
                for h in range(2):
                    nc.vector.tensor_reduce(
                        out=mh[h][:].rearrange("p (g o) -> p g o", o=1),
                        in_=h1T[:].rearrange("f (g h j) -> f g h j",
                                             h=2, j=25)[:, :, h, :],
                        axis=mybir.AxisListType.X, op=mybir.AluOpType.add)

            for k in range(3):
                pz = pm.tile([128, 2 * BL], F32, tag="ph", space="PSUM")
                nc.tensor.matmul(out=pz[:], lhsT=wh[k][0], rhs=h0T[:],
                                 start=True, stop=False)
                nc.tensor.matmul(out=pz[:], lhsT=wh[k][1], rhs=mh[0][:],
                                 start=False, stop=False)
                nc.tensor.matmul(out=pz[:], lhsT=wh[k][2], rhs=mh[1][:],
                                 start=False, stop=True)
                zh = fp.tile([128, 2 * BL], F16, tag="zh")
                nc.scalar.activation(out=zh[:], in_=pz[:], func=SIG)
                po = pm.tile([D, 2 * BL], F32, tag="po", space="PSUM")
                nc.tensor.matmul(out=po[:], lhsT=wd[k], rhs=zh[:],
                                 start=True, stop=True)
                ot = fp.tile([D, 2 * BL], F32, tag="ot")
                nc.vector.tensor_copy(out=ot[:], in_=po[:])
                nc.sync.dma_start(out=out_d[k, :, :], in_=ot[:])

    nc.compile()
    return nc


_NC_CACHE = None


def _get_nc():
    global _NC_CACHE
    if _NC_CACHE is None:
        _NC_CACHE = build_program()
    return _NC_CACHE


def _wrap16(ids, n):
    """Position-ordered ids -> [128, n//16] int16 wrapped, tiled 8x."""
    a = np.asarray(ids, dtype=np.int64)
    assert a.shape[0] == n and n % 16 == 0
    w = a.astype(np.int16).reshape(-1, 16).T  # [16, n//16]
    return np.ascontiguousarray(np.tile(w, (8, 1)))


def host_prep(nodes1, nodes2, neighbors_out, neighbors_in, features,
              W_in, W_mean, W_std, W_pi, Wd_mean, Wd_std, Wd_pi):
    nodes1 = np.asarray(nodes1, dtype=np.int64)
    nodes2 = np.asarray(nodes2, dtype=np.int64)
    nbr_out = np.asarray(neighbors_out, dtype=np.int64)
    nbr_in = np.asarray(neighbors_in, dtype=np.int64)
    feats = np.asarray(features, dtype=np.float32)

    def scale(w, f):
        w = np.array(w, dtype=np.float32, copy=True)
        w[F:] *= np.float32(f)
        return w.astype(np.float16)

    w1 = scale(W_in, 0.1)
    w0 = scale(W_in, 0.04)
    whs = [scale(W_mean, 0.04), scale(W_std, 0.04), scale(W_pi, 0.04)]
    wds = [np.asarray(w, dtype=np.float16) for w in (Wd_mean, Wd_std, Wd_pi)]

    c1o = (np.array(S1_C1_OUT), np.array(S2_C1_OUT))
    c1i = (np.array(S1_C1_IN), np.array(S2_C1_IN))
    c2o = (np.array(S1_C2_OUT), np.array(S2_C2_OUT))
    c2i = (np.array(S1_C2_IN), np.array(S2_C2_IN))

    in_maps = []
    for c in range(NCORES):
        # v-order: s, b, h1, j1  (concat(out25, in25) per base node)
        s1_parts = []
        base = np.concatenate([nodes1[c * BL:(c + 1) * BL],
                               nodes2[c * BL:(c + 1) * BL]])
        for s, nodes in ((0, nodes1), (1, nodes2)):
            bs = nodes[c * BL:(c + 1) * BL]
            no = nbr_out[bs][:, c1o[s]]           # [BL, 25]
            ni = nbr_in[bs][:, c1i[s]]            # [BL, 25]
            s1_parts.append(np.concatenate([no, ni], 1).reshape(-1))
        s1 = np.concatenate(s1_parts)             # [3200] v-order
        # hop-2 ids in (v, h2, j2) order
        h2o = np.concatenate([nbr_out[s1_parts[0]][:, c2o[0]],
                              nbr_out[s1_parts[1]][:, c2o[1]]])
        h2i = np.concatenate([nbr_in[s1_parts[0]][:, c2i[0]],
                              nbr_in[s1_parts[1]][:, c2i[1]]])
        ids2 = np.concatenate([h2o, h2i], 1).reshape(-1)  # [NV*20]

        # gather-position layouts
        v = np.arange(NV)
        R2 = np.empty(N2, dtype=np.int64)
        vv = np.repeat(v, 20)
        h2 = np.tile(np.repeat(np.arange(2), 10), NV)
        j2 = np.tile(np.arange(10), 2 * NV)
        pos2 = (vv % 128) + 128 * (((vv // 128) * 2 + h2) * 10 + j2)
        R2[pos2] = ids2
        R3 = np.full(NSB, -1, dtype=np.int64)
        R3[(v % 128) + 128 * (v // 128)] = s1
        R3[3200 + np.arange(2 * BL)] = base

        # 3 position segments, each with its own compacted table
        segs = [R2[:SEG[0]], R2[SEG[0]:SEG[0] + SEG[1]],
                np.concatenate([R2[SEG[0] + SEG[1]:], R3[:3264]])]
        tables = []
        locs = []
        for i, req in enumerate(segs):
            # first-use table order: first occurrences walk the table
            # near-sequentially -> HBM row-buffer friendly gathers
            uniq, first_idx, inv = np.unique(req, return_index=True,
                                             return_inverse=True)
            assert len(uniq) <= TROWS[i]
            order = np.argsort(first_idx)
            rank = np.empty_like(order)
            rank[order] = np.arange(len(order))
            table = np.zeros((TROWS[i], F), dtype=np.float16)
            table[:len(uniq)] = feats[uniq[order]].astype(np.float16)
            tables.append(table)
            locs.append(rank[inv])
        L2 = np.concatenate([locs[0], locs[1], locs[2][:SEG[2]]])
        L3 = np.full(NSB, -1, dtype=np.int64)
        L3[:3264] = locs[2][SEG[2]:]

        icat = np.hstack([_wrap16(L2, N2), _wrap16(L3, NSB)])
        blocks = [w1[q * 128:(q + 1) * 128] for q in range(3)]
        blocks += [w0[q * 128:(q + 1) * 128] for q in range(3)]
        for k in range(3):
            blocks += [whs[k][q * 128:(q + 1) * 128] for q in range(3)]
        blocks += wds
        wcat = np.ascontiguousarray(np.hstack(blocks))

        m = {"icat": icat, "wcat": wcat}
        for i in range(3):
            m[f"tab{i}"] = tables[i]
        in_maps.append(m)
    return in_maps


def kernel(nodes1, nodes2, neighbors_out, neighbors_in, features,
           W_in, W_mean, W_std, W_pi, W_ag, W_ad, Wd_mean, Wd_std, Wd_pi,
           _trace=False):
    in_maps = host_prep(nodes1, nodes2, neighbors_out, neighbors_in, features,
                        W_in, W_mean, W_std, W_pi, Wd_mean, Wd_std, Wd_pi)
    nc = _get_nc()
    from concourse.bass_utils import run_bass_kernel_spmd
    res = run_bass_kernel_spmd(nc, in_maps, list(range(NCORES)),
                               trace=_trace)
    if _trace:
        kernel.last_results = res

    out = np.zeros((6, B, D), dtype=np.float32)
    for c in range(NCORES):
        o = res.results[c]["out"]  # [3, D, 64] cols g = s*32+b
        for k in range(3):
            for s in range(2):
                out[s * 3 + k, c * BL:(c + 1) * BL, :] = \
                    o[k][:, s * BL:(s + 1) * BL].T
    return out


# revision 5
# speedup vs baseline: 1.1957x; 1.0198x over previous
"""Trainium2 Bass kernel for the DLSM GNN message-passing model, v2.

Data-parallel over the batch: each of 8 NeuronCores handles 32 nodes of
nodes1 + 32 of nodes2. The sampling indices (fixed RNG columns x input node
ids x input adjacency tables) are computed host-side; the device performs all
feature gathers, neighbor aggregation, and GC/head matmuls.

Key structure per core:
  - Host dedups the ~67k touched feature rows into a <=32768-row fp16 table
    (content-preserving relayout of `features`), so the batched SWDGE
    dma_gather (int16 indices, <=1024 per instruction, 4 queues) can fetch
    all rows in ~67 instructions instead of ~550 serial indirect DMAs.
  - Gather positions are laid out so hop-2 row j of slot (v,h) lands at
    [partition v%128, col (2*(v//128)+h)*10+j] -> neighbor sums are static
    strided DVE reduces; self rows land node-major [v%128, v//128].
  - Compute is fp16 in / f32 PSUM: per 128-node tile, PE-transpose self and
    the two neighbor sums to feature-major, 3-block GC matmul, ACT sigmoid.
    Hop-0 and the three heads reuse the transposed buffers.
"""
import numpy as np
import sys

sys.path.insert(0, '/opt/trn_rl_repo')

import concourse.bass as bass  # noqa: E402
import concourse.tile as tile  # noqa: E402
from concourse import bacc, mybir  # noqa: E402

# ---- problem constants -----------------------------------------------------
N = 200000
F = 128
B = 256
E = 128
D = 64
NCORES = 8
BL = B // NCORES            # 32 base nodes per side per core
NV = 2 * BL * 50            # 3200 hop-1 nodes per core (v-order: s,b,h1,j1)
NT = NV // 128              # 25 tiles
NSLOT = 50                  # hop-2 slots per partition (2 per tile)
N2 = NV * 20                # 64000 hop-2 gather positions
NSB = 3328                  # self(3200) + base(64) + pad(64) positions
GI = 1024                   # max idxs per dma_gather instruction
import os as _os
SINGLE_PACKET = _os.environ.get('K_SP', '1') == '1'
# hop-2 positions are split into 3 segments, each with its own compacted
# fp16 table (unique rows <= draws < 32768 so int16 indices always fit).
SEG = (25600, 25600, 12800)          # hop-2 positions per segment
TROWS = (25600, 25600, 12800 + 3264)  # table rows (seg2 also serves self+base)

F16 = mybir.dt.float16
F32 = mybir.dt.float32
I16 = mybir.dt.int16
SIG = mybir.ActivationFunctionType.Sigmoid
COPY = mybir.ActivationFunctionType.Copy

# Sampling columns fixed by jax.random.key(42) inside the reference.
S1_C1_OUT = [10, 56, 8, 17, 28, 26, 9, 20, 22, 35, 15, 4, 14, 21, 6, 53, 27,
             47, 49, 46, 41, 13, 63, 38, 54]
S1_C1_IN = [19, 59, 37, 12, 34, 31, 29, 1, 3, 0, 24, 40, 26, 11, 25, 23, 13,
            27, 43, 6, 57, 35, 58, 51, 9]
S1_C2_OUT = [57, 36, 9, 2, 34, 3, 6, 11, 0, 21]
S1_C2_IN = [33, 13, 21, 0, 54, 16, 46, 24, 30, 43]
S2_C1_OUT = [9, 7, 34, 52, 15, 35, 54, 30, 10, 16, 42, 56, 51, 28, 12, 19,
             24, 49, 2, 38, 43, 32, 48, 1, 39]
S2_C1_IN = [53, 47, 39, 57, 37, 27, 4, 20, 36, 31, 60, 38, 12, 43, 3, 21, 25,
            58, 48, 52, 23, 35, 15, 28, 7]
S2_C2_OUT = [41, 25, 9, 57, 45, 62, 42, 37, 31, 63]
S2_C2_IN = [40, 34, 60, 56, 2, 14, 6, 32, 50, 25]


def _gather_chunks(total):
    """Split `total` positions into <=GI chunks, each a multiple of 128."""
    out = []
    pos = 0
    while pos < total:
        n = min(GI, total - pos)
        out.append((pos, n))
        pos += n
    return out


def build_program():
    nc = bacc.Bacc("TRN2", target_bir_lowering=False, debug=False,
                   num_swdge_queues=4)

    tabs = [nc.dram_tensor(f"tab{i}", [TROWS[i], F], F16,
                           kind="ExternalInput") for i in range(3)]
    # all int16 index tensors concatenated: idx2 (4000 cols) + idxsb (208)
    icat_d = nc.dram_tensor("icat", [128, N2 // 16 + NSB // 16], I16,
                            kind="ExternalInput")
    # all weights concatenated: w1(3*128) w0(3*128) wh(9*128) wd(3*64) cols
    WCOLS = 15 * 128 + 3 * D + 128
    wcat_d = nc.dram_tensor("wcat", [128, WCOLS], F16, kind="ExternalInput")
    out_d = nc.dram_tensor("out", [3, D, 2 * BL], F32, kind="ExternalOutput")

    with tile.TileContext(nc) as tc:
        with (
            tc.tile_pool(name="const", bufs=1) as cp,
            tc.tile_pool(name="pers", bufs=1) as bp,
            tc.tile_pool(name="piece", bufs=3) as gp,
            tc.tile_pool(name="fmaj", bufs=4) as fp,
            tc.tile_pool(name="small", bufs=2) as sp_,
            tc.tile_pool(name="ps_t", bufs=2, space="PSUM") as pa,
            tc.tile_pool(name="ps_mm", bufs=1, space="PSUM") as pm,
        ):
            wcat = cp.tile([128, WCOLS], F16)
            nc.sync.dma_start(out=wcat[:], in_=wcat_d[:, :])
            w1 = [wcat[:, q * 128:(q + 1) * 128] for q in range(3)]
            w0 = [wcat[:, (3 + q) * 128:(4 + q) * 128] for q in range(3)]
            wh = [[wcat[:, (6 + k * 3 + q) * 128:(7 + k * 3 + q) * 128]
                   for q in range(3)] for k in range(3)]
            wd = [wcat[:, 15 * 128 + k * D:15 * 128 + (k + 1) * D]
                  for k in range(3)]
            ident = wcat[:, 15 * 128 + 3 * D:]

            icat = cp.tile([128, N2 // 16 + NSB // 16], I16)
            nc.sync.dma_start(out=icat[:], in_=icat_d[:, :])
            idx2 = icat[:, 0:N2 // 16]
            idxsb = icat[:, N2 // 16:]

            # ---- self + base feature gather (node-major [q, v//128, f]) ---
            fs = bp.tile([128, (NSB // 128) * F], F16, tag="fs")
            fs3 = fs[:].rearrange("p (c f) -> p c f", f=F)
            qn = 0
            for pos, n in _gather_chunks(NSB):
                nvalid = min(n, 3264 - pos)
                c0 = pos // 128
                nc.gpsimd.dma_gather(
                    fs3[:, c0:c0 + n // 128, :], tabs[2][:, :],
                    idxsb[:, pos // 16:(pos + n) // 16],
                    n, nvalid, F, queue_num=qn,
                    single_packet=SINGLE_PACKET)
                qn = (qn + 1) % 4

            # persistent buffers
            fselfT = bp.tile([128, NV], F16, tag="fselfT")
            h1T = bp.tile([128, NV], F16, tag="h1T")
            msum = bp.tile([128, NSLOT * F], F16, tag="msum")
            msum3 = msum[:].rearrange("p (s f) -> p s f", f=F)

            # ---- front-loaded: self transposes, base transpose, hop-0 -----
            for t in range(NV // 128):
                ps_s = pa.tile([128, 128], F16, tag="ps_s", space="PSUM")
                nc.tensor.matmul(out=ps_s[:], lhsT=fs3[:, t, :], rhs=ident,
                                 start=True, stop=True, is_transpose=True)
                nc.scalar.activation(out=fselfT[:, t * 128:(t + 1) * 128],
                                     in_=ps_s[:], func=COPY)
            ps_b = pa.tile([128, 128], F16, tag="ps_s", space="PSUM")
            nc.tensor.matmul(out=ps_b[:], lhsT=fs3[:, 25, :], rhs=ident,
                             start=True, stop=True, is_transpose=True)
            fbT = sp_.tile([128, 128], F16, tag="fbT")
            nc.scalar.activation(out=fbT[:], in_=ps_b[:], func=COPY)

            m0 = [sp_.tile([128, 64], F16, tag=f"m0_{h}", name=f"m0_{h}")
                  for h in range(2)]
            mh = [sp_.tile([128, 64], F16, tag=f"mh_{h}", name=f"mh_{h}")
                  for h in range(2)]
            with nc.allow_low_precision("fp16 means, tol 2e-2"):
                for h in range(2):
                    nc.vector.tensor_reduce(
                        out=m0[h][:].rearrange("p (g o) -> p g o", o=1),
                        in_=fselfT[:].rearrange("f (g h j) -> f g h j",
                                                h=2, j=25)[:, :, h, :],
                        axis=mybir.AxisListType.X, op=mybir.AluOpType.add)

            ph0 = pm.tile([128, 2 * BL], F32, tag="ph", space="PSUM")
            nc.tensor.matmul(out=ph0[:], lhsT=w0[0], rhs=fbT[:, 0:2 * BL],
                             start=True, stop=False)
            nc.tensor.matmul(out=ph0[:], lhsT=w0[1], rhs=m0[0][:],
                             start=False, stop=False)
            nc.tensor.matmul(out=ph0[:], lhsT=w0[2], rhs=m0[1][:],
                             start=False, stop=True)
            h0T = sp_.tile([128, 2 * BL], F16, tag="h0T")
            nc.scalar.activation(out=h0T[:], in_=ph0[:], func=SIG)

            # ---- hop-2 pipeline: pieces of 2 tiles (40 cols, 5120 idxs) ---
            pieces = []
            pos = 0
            while pos < N2:
                n = min(5120, N2 - pos)
                pieces.append((pos, n))
                pos += n
            for pos, n in pieces:
                seg = 0 if pos < SEG[0] else (1 if pos < SEG[0] + SEG[1]
                                              else 2)
                ptile = gp.tile([128, (n // 128) * F], F16, tag="piece")
                p3 = ptile[:].rearrange("p (c f) -> p c f", f=F)
                for off, gn in _gather_chunks(n):
                    c0 = off // 128
                    nc.gpsimd.dma_gather(
                        p3[:, c0:c0 + gn // 128, :], tabs[seg][:, :],
                        idx2[:, (pos + off) // 16:(pos + off + gn) // 16],
                        gn, gn, F, queue_num=qn,
                        single_packet=SINGLE_PACKET)
                    qn = (qn + 1) % 4
                # neighbor sums: contiguous-run add tree (10 -> 5 -> 2+1)
                s0 = (pos // 128) // 10
                ns = (n // 128) // 10
                x4 = ptile[:].rearrange("p (s j f) -> p s j f", j=10, f=F)
                T = gp.tile([128, ns * 5 * F], F16, tag="redT",
                            name=f"redT_{pos}")
                t4 = T[:].rearrange("p (s j f) -> p s j f", j=5, f=F)
                U = gp.tile([128, ns * 2 * F], F16, tag="redU",
                            name=f"redU_{pos}")
                u4 = U[:].rearrange("p (s j f) -> p s j f", j=2, f=F)
                with nc.allow_low_precision("fp16 neighbor sums, tol 2e-2"):
                    nc.vector.tensor_add(out=t4, in0=x4[:, :, 0:5, :],
                                         in1=x4[:, :, 5:10, :])
                    nc.vector.tensor_add(out=u4, in0=t4[:, :, 0:2, :],
                                         in1=t4[:, :, 2:4, :])
                    nc.vector.tensor_add(out=u4[:, :, 0, :],
                                         in0=u4[:, :, 0, :],
                                         in1=u4[:, :, 1, :])
                    nc.vector.tensor_add(out=msum3[:, s0:s0 + ns, :],
                                         in0=u4[:, :, 0, :],
                                         in1=t4[:, :, 4, :])

                # GC for the tiles completed by this piece
                for t in range(s0 // 2, s0 // 2 + ns // 2):
                    ps_o = pa.tile([128, 128], F16, tag="ps_o", space="PSUM")
                    ps_i = pa.tile([128, 128], F16, tag="ps_i", space="PSUM")
                    nc.tensor.matmul(out=ps_o[:], lhsT=msum3[:, 2 * t, :],
                                     rhs=ident, start=True, stop=True,
                                     is_transpose=True)
                    nc.tensor.matmul(out=ps_i[:], lhsT=msum3[:, 2 * t + 1, :],
                                     rhs=ident, start=True, stop=True,
                                     is_transpose=True)
                    so = fp.tile([128, 128], F16, tag="so")
                    si = fp.tile([128, 128], F16, tag="si")
                    nc.scalar.activation(out=so[:], in_=ps_o[:], func=COPY)
                    nc.scalar.activation(out=si[:], in_=ps_i[:], func=COPY)

                    ph = pm.tile([128, 128], F32, tag="ph", space="PSUM")
                    nc.tensor.matmul(out=ph[:], lhsT=w1[0],
                                     rhs=fselfT[:, t * 128:(t + 1) * 128],
                                     start=True, stop=False)
                    nc.tensor.matmul(out=ph[:], lhsT=w1[1], rhs=so[:],
                                     start=False, stop=False)
                    nc.tensor.matmul(out=ph[:], lhsT=w1[2], rhs=si[:],
                                     start=False, stop=True)
                    nc.scalar.activation(out=h1T[:, t * 128:(t + 1) * 128],
                                         in_=ph[:], func=SIG)

            # ---- layer-1 means + heads (short tail) -----------------------
            with nc.allow_low_precision("fp16 means, tol 2e-2"):
                for h in range(2):
                    nc.vector.tensor_reduce(
                        out=mh[h][:].rearrange("p (g o) -> p g o", o=1),
                        in_=h1T[:].rearrange("f (g h j) -> f g h j",
                                             h=2, j=25)[:, :, h, :],
                        axis=mybir.AxisListType.X, op=mybir.AluOpType.add)

            for k in range(3):
                pz = pm.tile([128, 2 * BL], F32, tag="ph", space="PSUM")
                nc.tensor.matmul(out=pz[:], lhsT=wh[k][0], rhs=h0T[:],
                                 start=True, stop=False)
                nc.tensor.matmul(out=pz[:], lhsT=wh[k][1], rhs=mh[0][:],
                                 start=False, stop=False)
                nc.tensor.matmul(out=pz[:], lhsT=wh[k][2], rhs=mh[1][:],
                                 start=False, stop=True)
                zh = fp.tile([128, 2 * BL], F16, tag="zh")
                nc.scalar.activation(out=zh[:], in_=pz[:], func=SIG)
                po = pm.tile([D, 2 * BL], F32, tag="po", space="PSUM")
                nc.tensor.matmul(out=po[:], lhsT=wd[k], rhs=zh[:],
                                 start=True, stop=True)
                ot = fp.tile([D, 2 * BL], F32, tag="ot")
                nc.vector.tensor_copy(out=ot[:], in_=po[:])
                nc.sync.dma_start(out=out_d[k, :, :], in_=ot[:])

    nc.compile()
    return nc


_NC_CACHE = None


def _get_nc():
    global _NC_CACHE
    if _NC_CACHE is None:
        _NC_CACHE = build_program()
    return _NC_CACHE


def _wrap16(ids, n):
    """Position-ordered ids -> [128, n//16] int16 wrapped, tiled 8x."""
    a = np.asarray(ids, dtype=np.int64)
    assert a.shape[0] == n and n % 16 == 0
    w = a.astype(np.int16).reshape(-1, 16).T  # [16, n//16]
    return np.ascontiguousarray(np.tile(w, (8, 1)))


def host_prep(nodes1, nodes2, neighbors_out, neighbors_in, features,
              W_in, W_mean, W_std, W_pi, Wd_mean, Wd_std, Wd_pi):
    nodes1 = np.asarray(nodes1, dtype=np.int64)
    nodes2 = np.asarray(nodes2, dtype=np.int64)
    nbr_out = np.asarray(neighbors_out, dtype=np.int64)
    nbr_in = np.asarray(neighbors_in, dtype=np.int64)
    feats = np.asarray(features, dtype=np.float32)

    def scale(w, f):
        w = np.array(w, dtype=np.float32, copy=True)
        w[F:] *= np.float32(f)
        return w.astype(np.float16)

    w1 = scale(W_in, 0.1)
    w0 = scale(W_in, 0.04)
    whs = [scale(W_mean, 0.04), scale(W_std, 0.04), scale(W_pi, 0.04)]
    wds = [np.asarray(w, dtype=np.float16) for w in (Wd_mean, Wd_std, Wd_pi)]

    c1o = (np.array(S1_C1_OUT), np.array(S2_C1_OUT))
    c1i = (np.array(S1_C1_IN), np.array(S2_C1_IN))
    c2o = (np.array(S1_C2_OUT), np.array(S2_C2_OUT))
    c2i = (np.array(S1_C2_IN), np.array(S2_C2_IN))

    in_maps = []
    for c in range(NCORES):
        # v-order: s, b, h1, j1  (concat(out25, in25) per base node)
        s1_parts = []
        base = np.concatenate([nodes1[c * BL:(c + 1) * BL],
                               nodes2[c * BL:(c + 1) * BL]])
        for s, nodes in ((0, nodes1), (1, nodes2)):
            bs = nodes[c * BL:(c + 1) * BL]
            no = nbr_out[bs][:, c1o[s]]           # [BL, 25]
            ni = nbr_in[bs][:, c1i[s]]            # [BL, 25]
            s1_parts.append(np.concatenate([no, ni], 1).reshape(-1))
        s1 = np.concatenate(s1_parts)             # [3200] v-order
        # hop-2 ids in (v, h2, j2) order
        h2o = np.concatenate([nbr_out[s1_parts[0]][:, c2o[0]],
                              nbr_out[s1_parts[1]][:, c2o[1]]])
        h2i = np.concatenate([nbr_in[s1_parts[0]][:, c2i[0]],
                              nbr_in[s1_parts[1]][:, c2i[1]]])
        ids2 = np.concatenate([h2o, h2i], 1).reshape(-1)  # [NV*20]

        # gather-position layouts
        v = np.arange(NV)
        R2 = np.empty(N2, dtype=np.int64)
        vv = np.repeat(v, 20)
        h2 = np.tile(np.repeat(np.arange(2), 10), NV)
        j2 = np.tile(np.arange(10), 2 * NV)
        pos2 = (vv % 128) + 128 * (((vv // 128) * 2 + h2) * 10 + j2)
        R2[pos2] = ids2
        R3 = np.full(NSB, -1, dtype=np.int64)
        R3[(v % 128) + 128 * (v // 128)] = s1
        R3[3200 + np.arange(2 * BL)] = base

        # 3 position segments, each with its own compacted table
        segs = [R2[:SEG[0]], R2[SEG[0]:SEG[0] + SEG[1]],
                np.concatenate([R2[SEG[0] + SEG[1]:], R3[:3264]])]
        tables = []
        locs = []
        for i, req in enumerate(segs):
            # first-use table order: first occurrences walk the table
            # near-sequentially -> HBM row-buffer friendly gathers
            uniq, first_idx, inv = np.unique(req, return_index=True,
                                             return_inverse=True)
            assert len(uniq) <= TROWS[i]
            order = np.argsort(first_idx)
            rank = np.empty_like(order)
            rank[order] = np.arange(len(order))
            table = np.zeros((TROWS[i], F), dtype=np.float16)
            table[:len(uniq)] = feats[uniq[order]].astype(np.float16)
            tables.append(table)
            locs.append(rank[inv])
        L2 = np.concatenate([locs[0], locs[1], locs[2][:SEG[2]]])
        L3 = np.full(NSB, -1, dtype=np.int64)
        L3[:3264] = locs[2][SEG[2]:]

        icat = np.hstack([_wrap16(L2, N2), _wrap16(L3, NSB)])
        blocks = [w1[q * 128:(q + 1) * 128] for q in range(3)]
        blocks += [w0[q * 128:(q + 1) * 128] for q in range(3)]
        for k in range(3):
            blocks += [whs[k][q * 128:(q + 1) * 128] for q in range(3)]
        blocks += wds
        blocks.append(np.eye(128, dtype=np.float16))
        wcat = np.ascontiguousarray(np.hstack(blocks))

        m = {"icat": icat, "wcat": wcat}
        for i in range(3):
            m[f"tab{i}"] = tables[i]
        in_maps.append(m)
    return in_maps


def kernel(nodes1, nodes2, neighbors_out, neighbors_in, features,
           W_in, W_mean, W_std, W_pi, W_ag, W_ad, Wd_mean, Wd_std, Wd_pi,
           _trace=False):
    in_maps = host_prep(nodes1, nodes2, neighbors_out, neighbors_in, features,
                        W_in, W_mean, W_std, W_pi, Wd_mean, Wd_std, Wd_pi)
    nc = _get_nc()
    from concourse.bass_utils import run_bass_kernel_spmd
    res = run_bass_kernel_spmd(nc, in_maps, list(range(NCORES)),
                               trace=_trace)
    if _trace:
        kernel.last_results = res

    out = np.zeros((6, B, D), dtype=np.float32)
    for c in range(NCORES):
        o = res.results[c]["out"]  # [3, D, 64] cols g = s*32+b
        for k in range(3):
            for s in range(2):
                out[s * 3 + k, c * BL:(c + 1) * BL, :] = \
                    o[k][:, s * BL:(s + 1) * BL].T
    return out


# revision 6
# speedup vs baseline: 1.2042x; 1.0071x over previous
"""Trainium2 Bass kernel for the DLSM GNN message-passing model, v2.

Data-parallel over the batch: each of 8 NeuronCores handles 32 nodes of
nodes1 + 32 of nodes2. The sampling indices (fixed RNG columns x input node
ids x input adjacency tables) are computed host-side; the device performs all
feature gathers, neighbor aggregation, and GC/head matmuls.

Key structure per core:
  - Host dedups the ~67k touched feature rows into a <=32768-row fp16 table
    (content-preserving relayout of `features`), so the batched SWDGE
    dma_gather (int16 indices, <=1024 per instruction, 4 queues) can fetch
    all rows in ~67 instructions instead of ~550 serial indirect DMAs.
  - Gather positions are laid out so hop-2 row j of slot (v,h) lands at
    [partition v%128, col (2*(v//128)+h)*10+j] -> neighbor sums are static
    strided DVE reduces; self rows land node-major [v%128, v//128].
  - Compute is fp16 in / f32 PSUM: per 128-node tile, PE-transpose self and
    the two neighbor sums to feature-major, 3-block GC matmul, ACT sigmoid.
    Hop-0 and the three heads reuse the transposed buffers.
"""
import numpy as np
import sys

sys.path.insert(0, '/opt/trn_rl_repo')

import concourse.bass as bass  # noqa: E402
import concourse.tile as tile  # noqa: E402
from concourse import bacc, mybir  # noqa: E402

# ---- problem constants -----------------------------------------------------
N = 200000
F = 128
B = 256
E = 128
D = 64
NCORES = 8
BL = B // NCORES            # 32 base nodes per side per core
NV = 2 * BL * 50            # 3200 hop-1 nodes per core (v-order: s,b,h1,j1)
NT = NV // 128              # 25 tiles
NSLOT = 50                  # hop-2 slots per partition (2 per tile)
N2 = NV * 20                # 64000 hop-2 gather positions
NSB = 3328                  # self(3200) + base(64) + pad(64) positions
GI = 1024                   # max idxs per dma_gather instruction
import os as _os
SINGLE_PACKET = _os.environ.get('K_SP', '0') == '1'
# hop-2 positions are split into 3 segments, each with its own compacted
# fp16 table (unique rows <= draws < 32768 so int16 indices always fit).
SEG = (25600, 25600, 12800)          # hop-2 positions per segment
TROWS = (25600, 25600, 12800 + 3264)  # table rows (seg2 also serves self+base)

F16 = mybir.dt.float16
F32 = mybir.dt.float32
I16 = mybir.dt.int16
SIG = mybir.ActivationFunctionType.Sigmoid
COPY = mybir.ActivationFunctionType.Copy

# Sampling columns fixed by jax.random.key(42) inside the reference.
S1_C1_OUT = [10, 56, 8, 17, 28, 26, 9, 20, 22, 35, 15, 4, 14, 21, 6, 53, 27,
             47, 49, 46, 41, 13, 63, 38, 54]
S1_C1_IN = [19, 59, 37, 12, 34, 31, 29, 1, 3, 0, 24, 40, 26, 11, 25, 23, 13,
            27, 43, 6, 57, 35, 58, 51, 9]
S1_C2_OUT = [57, 36, 9, 2, 34, 3, 6, 11, 0, 21]
S1_C2_IN = [33, 13, 21, 0, 54, 16, 46, 24, 30, 43]
S2_C1_OUT = [9, 7, 34, 52, 15, 35, 54, 30, 10, 16, 42, 56, 51, 28, 12, 19,
             24, 49, 2, 38, 43, 32, 48, 1, 39]
S2_C1_IN = [53, 47, 39, 57, 37, 27, 4, 20, 36, 31, 60, 38, 12, 43, 3, 21, 25,
            58, 48, 52, 23, 35, 15, 28, 7]
S2_C2_OUT = [41, 25, 9, 57, 45, 62, 42, 37, 31, 63]
S2_C2_IN = [40, 34, 60, 56, 2, 14, 6, 32, 50, 25]


def _gather_chunks(total):
    """Split `total` positions into <=GI chunks, each a multiple of 128."""
    out = []
    pos = 0
    while pos < total:
        n = min(GI, total - pos)
        out.append((pos, n))
        pos += n
    return out


def build_program():
    nc = bacc.Bacc("TRN2", target_bir_lowering=False, debug=False,
                   num_swdge_queues=4)

    tabs = [nc.dram_tensor(f"tab{i}", [TROWS[i], F], F16,
                           kind="ExternalInput") for i in range(3)]
    # all int16 index tensors concatenated: idx2 (4000 cols) + idxsb (208)
    icat_d = nc.dram_tensor("icat", [128, N2 // 16 + NSB // 16], I16,
                            kind="ExternalInput")
    # all weights concatenated: w1(3*128) w0(3*128) wh(9*128) wd(3*64) cols
    WCOLS = 15 * 128 + 3 * D + 128
    wcat_d = nc.dram_tensor("wcat", [128, WCOLS], F16, kind="ExternalInput")
    out_d = nc.dram_tensor("out", [3, D, 2 * BL], F32, kind="ExternalOutput")

    with tile.TileContext(nc) as tc:
        with (
            tc.tile_pool(name="const", bufs=1) as cp,
            tc.tile_pool(name="pers", bufs=1) as bp,
            tc.tile_pool(name="piece", bufs=3) as gp,
            tc.tile_pool(name="fmaj", bufs=4) as fp,
            tc.tile_pool(name="small", bufs=2) as sp_,
            tc.tile_pool(name="ps_t", bufs=3, space="PSUM") as pa,
            tc.tile_pool(name="ps_mm", bufs=2, space="PSUM") as pm,
        ):
            wcat = cp.tile([128, WCOLS], F16)
            nc.sync.dma_start(out=wcat[:], in_=wcat_d[:, :])
            w1 = [wcat[:, q * 128:(q + 1) * 128] for q in range(3)]
            w0 = [wcat[:, (3 + q) * 128:(4 + q) * 128] for q in range(3)]
            wh = [[wcat[:, (6 + k * 3 + q) * 128:(7 + k * 3 + q) * 128]
                   for q in range(3)] for k in range(3)]
            wd = [wcat[:, 15 * 128 + k * D:15 * 128 + (k + 1) * D]
                  for k in range(3)]
            ident = wcat[:, 15 * 128 + 3 * D:]

            icat = cp.tile([128, N2 // 16 + NSB // 16], I16)
            nc.sync.dma_start(out=icat[:], in_=icat_d[:, :])
            idx2 = icat[:, 0:N2 // 16]
            idxsb = icat[:, N2 // 16:]

            # ---- self + base feature gather (node-major [q, v//128, f]) ---
            fs = bp.tile([128, (NSB // 128) * F], F16, tag="fs")
            fs3 = fs[:].rearrange("p (c f) -> p c f", f=F)
            qn = 0
            for pos, n in _gather_chunks(NSB):
                nvalid = min(n, 3264 - pos)
                c0 = pos // 128
                nc.gpsimd.dma_gather(
                    fs3[:, c0:c0 + n // 128, :], tabs[2][:, :],
                    idxsb[:, pos // 16:(pos + n) // 16],
                    n, nvalid, F, queue_num=qn,
                    single_packet=SINGLE_PACKET)
                qn = (qn + 1) % 4

            # persistent buffers
            fselfT = bp.tile([128, NV], F16, tag="fselfT")
            h1T = bp.tile([128, NV], F16, tag="h1T")
            msum = bp.tile([128, NSLOT * F], F16, tag="msum")
            msum3 = msum[:].rearrange("p (s f) -> p s f", f=F)

            # ---- front-loaded: self transposes, base transpose, hop-0 -----
            for t in range(NV // 128):
                ps_s = pa.tile([128, 128], F16, tag="ps_s", space="PSUM")
                nc.tensor.matmul(out=ps_s[:], lhsT=fs3[:, t, :], rhs=ident,
                                 start=True, stop=True, is_transpose=True)
                nc.scalar.activation(out=fselfT[:, t * 128:(t + 1) * 128],
                                     in_=ps_s[:], func=COPY)
            ps_b = pa.tile([128, 128], F16, tag="ps_s", space="PSUM")
            nc.tensor.matmul(out=ps_b[:], lhsT=fs3[:, 25, :], rhs=ident,
                             start=True, stop=True, is_transpose=True)
            fbT = sp_.tile([128, 128], F16, tag="fbT")
            nc.scalar.activation(out=fbT[:], in_=ps_b[:], func=COPY)

            m0 = [sp_.tile([128, 64], F16, tag=f"m0_{h}", name=f"m0_{h}")
                  for h in range(2)]
            mh = [sp_.tile([128, 64], F16, tag=f"mh_{h}", name=f"mh_{h}")
                  for h in range(2)]
            with nc.allow_low_precision("fp16 means, tol 2e-2"):
                for h in range(2):
                    nc.vector.tensor_reduce(
                        out=m0[h][:].rearrange("p (g o) -> p g o", o=1),
                        in_=fselfT[:].rearrange("f (g h j) -> f g h j",
                                                h=2, j=25)[:, :, h, :],
                        axis=mybir.AxisListType.X, op=mybir.AluOpType.add)

            ph0 = pm.tile([128, 2 * BL], F32, tag="ph", space="PSUM")
            nc.tensor.matmul(out=ph0[:], lhsT=w0[0], rhs=fbT[:, 0:2 * BL],
                             start=True, stop=False)
            nc.tensor.matmul(out=ph0[:], lhsT=w0[1], rhs=m0[0][:],
                             start=False, stop=False)
            nc.tensor.matmul(out=ph0[:], lhsT=w0[2], rhs=m0[1][:],
                             start=False, stop=True)
            h0T = sp_.tile([128, 2 * BL], F16, tag="h0T")
            nc.scalar.activation(out=h0T[:], in_=ph0[:], func=SIG)

            # ---- hop-2 pipeline: pieces of 2 tiles (40 cols, 5120 idxs) ---
            pieces = []
            pos = 0
            while pos < N2:
                n = min(5120, N2 - pos)
                pieces.append((pos, n))
                pos += n
            for pos, n in pieces:
                seg = 0 if pos < SEG[0] else (1 if pos < SEG[0] + SEG[1]
                                              else 2)
                ptile = gp.tile([128, (n // 128) * F], F16, tag="piece")
                p3 = ptile[:].rearrange("p (c f) -> p c f", f=F)
                for off, gn in _gather_chunks(n):
                    c0 = off // 128
                    nc.gpsimd.dma_gather(
                        p3[:, c0:c0 + gn // 128, :], tabs[seg][:, :],
                        idx2[:, (pos + off) // 16:(pos + off + gn) // 16],
                        gn, gn, F, queue_num=qn,
                        single_packet=SINGLE_PACKET)
                    qn = (qn + 1) % 4
                # neighbor sums: contiguous-run add tree (10 -> 5 -> 2+1)
                s0 = (pos // 128) // 10
                ns = (n // 128) // 10
                x4 = ptile[:].rearrange("p (s j f) -> p s j f", j=10, f=F)
                T = gp.tile([128, ns * 5 * F], F16, tag="redT",
                            name=f"redT_{pos}")
                t4 = T[:].rearrange("p (s j f) -> p s j f", j=5, f=F)
                U = gp.tile([128, ns * 2 * F], F16, tag="redU",
                            name=f"redU_{pos}")
                u4 = U[:].rearrange("p (s j f) -> p s j f", j=2, f=F)
                with nc.allow_low_precision("fp16 neighbor sums, tol 2e-2"):
                    nc.vector.tensor_add(out=t4, in0=x4[:, :, 0:5, :],
                                         in1=x4[:, :, 5:10, :])
                    nc.vector.tensor_add(out=u4, in0=t4[:, :, 0:2, :],
                                         in1=t4[:, :, 2:4, :])
                    nc.vector.tensor_add(out=u4[:, :, 0, :],
                                         in0=u4[:, :, 0, :],
                                         in1=u4[:, :, 1, :])
                    nc.vector.tensor_add(out=msum3[:, s0:s0 + ns, :],
                                         in0=u4[:, :, 0, :],
                                         in1=t4[:, :, 4, :])

                # GC for the tiles completed by this piece
                for t in range(s0 // 2, s0 // 2 + ns // 2):
                    ps_o = pa.tile([128, 128], F16, tag="ps_s", space="PSUM")
                    ps_i = pa.tile([128, 128], F16, tag="ps_s", space="PSUM")
                    nc.tensor.matmul(out=ps_o[:], lhsT=msum3[:, 2 * t, :],
                                     rhs=ident, start=True, stop=True,
                                     is_transpose=True)
                    nc.tensor.matmul(out=ps_i[:], lhsT=msum3[:, 2 * t + 1, :],
                                     rhs=ident, start=True, stop=True,
                                     is_transpose=True)
                    so = fp.tile([128, 128], F16, tag="so")
                    si = fp.tile([128, 128], F16, tag="si")
                    nc.scalar.activation(out=so[:], in_=ps_o[:], func=COPY)
                    nc.scalar.activation(out=si[:], in_=ps_i[:], func=COPY)

                    ph = pm.tile([128, 128], F32, tag="ph", space="PSUM")
                    nc.tensor.matmul(out=ph[:], lhsT=w1[0],
                                     rhs=fselfT[:, t * 128:(t + 1) * 128],
                                     start=True, stop=False)
                    nc.tensor.matmul(out=ph[:], lhsT=w1[1], rhs=so[:],
                                     start=False, stop=False)
                    nc.tensor.matmul(out=ph[:], lhsT=w1[2], rhs=si[:],
                                     start=False, stop=True)
                    nc.scalar.activation(out=h1T[:, t * 128:(t + 1) * 128],
                                         in_=ph[:], func=SIG)

            # ---- layer-1 means + heads (short tail) -----------------------
            with nc.allow_low_precision("fp16 means, tol 2e-2"):
                for h in range(2):
                    nc.vector.tensor_reduce(
                        out=mh[h][:].rearrange("p (g o) -> p g o", o=1),
                        in_=h1T[:].rearrange("f (g h j) -> f g h j",
                                             h=2, j=25)[:, :, h, :],
                        axis=mybir.AxisListType.X, op=mybir.AluOpType.add)

            oall = sp_.tile([D, 3 * 2 * BL], F32, tag="oall")
            oall3 = oall[:].rearrange("d (k g) -> d k g", g=2 * BL)
            for k in range(3):
                pz = pm.tile([128, 2 * BL], F32, tag="ph", space="PSUM")
                nc.tensor.matmul(out=pz[:], lhsT=wh[k][0], rhs=h0T[:],
                                 start=True, stop=False)
                nc.tensor.matmul(out=pz[:], lhsT=wh[k][1], rhs=mh[0][:],
                                 start=False, stop=False)
                nc.tensor.matmul(out=pz[:], lhsT=wh[k][2], rhs=mh[1][:],
                                 start=False, stop=True)
                zh = fp.tile([128, 2 * BL], F16, tag="zh")
                nc.scalar.activation(out=zh[:], in_=pz[:], func=SIG)
                po = pm.tile([D, 2 * BL], F32, tag="po", space="PSUM")
                nc.tensor.matmul(out=po[:], lhsT=wd[k], rhs=zh[:],
                                 start=True, stop=True)
                nc.vector.tensor_copy(out=oall3[:, k, :], in_=po[:])
            nc.sync.dma_start(
                out=out_d[:, :, :].rearrange("k d g -> d k g"),
                in_=oall3)

    nc.compile()
    return nc


_NC_CACHE = None


def _get_nc():
    global _NC_CACHE
    if _NC_CACHE is None:
        _NC_CACHE = build_program()
    return _NC_CACHE


def _wrap16(ids, n):
    """Position-ordered ids -> [128, n//16] int16 wrapped, tiled 8x."""
    a = np.asarray(ids, dtype=np.int64)
    assert a.shape[0] == n and n % 16 == 0
    w = a.astype(np.int16).reshape(-1, 16).T  # [16, n//16]
    return np.ascontiguousarray(np.tile(w, (8, 1)))


def host_prep(nodes1, nodes2, neighbors_out, neighbors_in, features,
              W_in, W_mean, W_std, W_pi, Wd_mean, Wd_std, Wd_pi):
    nodes1 = np.asarray(nodes1, dtype=np.int64)
    nodes2 = np.asarray(nodes2, dtype=np.int64)
    nbr_out = np.asarray(neighbors_out, dtype=np.int64)
    nbr_in = np.asarray(neighbors_in, dtype=np.int64)
    feats = np.asarray(features, dtype=np.float32)

    def scale(w, f):
        w = np.array(w, dtype=np.float32, copy=True)
        w[F:] *= np.float32(f)
        return w.astype(np.float16)

    w1 = scale(W_in, 0.1)
    w0 = scale(W_in, 0.04)
    whs = [scale(W_mean, 0.04), scale(W_std, 0.04), scale(W_pi, 0.04)]
    wds = [np.asarray(w, dtype=np.float16) for w in (Wd_mean, Wd_std, Wd_pi)]

    c1o = (np.array(S1_C1_OUT), np.array(S2_C1_OUT))
    c1i = (np.array(S1_C1_IN), np.array(S2_C1_IN))
    c2o = (np.array(S1_C2_OUT), np.array(S2_C2_OUT))
    c2i = (np.array(S1_C2_IN), np.array(S2_C2_IN))

    in_maps = []
    for c in range(NCORES):
        # v-order: s, b, h1, j1  (concat(out25, in25) per base node)
        s1_parts = []
        base = np.concatenate([nodes1[c * BL:(c + 1) * BL],
                               nodes2[c * BL:(c + 1) * BL]])
        for s, nodes in ((0, nodes1), (1, nodes2)):
            bs = nodes[c * BL:(c + 1) * BL]
            no = nbr_out[bs][:, c1o[s]]           # [BL, 25]
            ni = nbr_in[bs][:, c1i[s]]            # [BL, 25]
            s1_parts.append(np.concatenate([no, ni], 1).reshape(-1))
        s1 = np.concatenate(s1_parts)             # [3200] v-order
        # hop-2 ids in (v, h2, j2) order
        h2o = np.concatenate([nbr_out[s1_parts[0]][:, c2o[0]],
                              nbr_out[s1_parts[1]][:, c2o[1]]])
        h2i = np.concatenate([nbr_in[s1_parts[0]][:, c2i[0]],
                              nbr_in[s1_parts[1]][:, c2i[1]]])
        ids2 = np.concatenate([h2o, h2i], 1).reshape(-1)  # [NV*20]

        # gather-position layouts
        v = np.arange(NV)
        R2 = np.empty(N2, dtype=np.int64)
        vv = np.repeat(v, 20)
        h2 = np.tile(np.repeat(np.arange(2), 10), NV)
        j2 = np.tile(np.arange(10), 2 * NV)
        pos2 = (vv % 128) + 128 * (((vv // 128) * 2 + h2) * 10 + j2)
        R2[pos2] = ids2
        R3 = np.full(NSB, -1, dtype=np.int64)
        R3[(v % 128) + 128 * (v // 128)] = s1
        R3[3200 + np.arange(2 * BL)] = base

        # 3 position segments, each with its own compacted table
        segs = [R2[:SEG[0]], R2[SEG[0]:SEG[0] + SEG[1]],
                np.concatenate([R2[SEG[0] + SEG[1]:], R3[:3264]])]
        tables = []
        locs = []
        for i, req in enumerate(segs):
            # first-use table order: first occurrences walk the table
            # near-sequentially -> HBM row-buffer friendly gathers
            uniq, first_idx, inv = np.unique(req, return_index=True,
                                             return_inverse=True)
            assert len(uniq) <= TROWS[i]
            order = np.argsort(first_idx)
            rank = np.empty_like(order)
            rank[order] = np.arange(len(order))
            table = np.zeros((TROWS[i], F), dtype=np.float16)
            table[:len(uniq)] = feats[uniq[order]].astype(np.float16)
            tables.append(table)
            locs.append(rank[inv])
        L2 = np.concatenate([locs[0], locs[1], locs[2][:SEG[2]]])
        L3 = np.full(NSB, -1, dtype=np.int64)
        L3[:3264] = locs[2][SEG[2]:]

        icat = np.hstack([_wrap16(L2, N2), _wrap16(L3, NSB)])
        blocks = [w1[q * 128:(q + 1) * 128] for q in range(3)]
        blocks += [w0[q * 128:(q + 1) * 128] for q in range(3)]
        for k in range(3):
            blocks += [whs[k][q * 128:(q + 1) * 128] for q in range(3)]
        blocks += wds
        blocks.append(np.eye(128, dtype=np.float16))
        wcat = np.ascontiguousarray(np.hstack(blocks))

        m = {"icat": icat, "wcat": wcat}
        for i in range(3):
            m[f"tab{i}"] = tables[i]
        in_maps.append(m)
    return in_maps


def kernel(nodes1, nodes2, neighbors_out, neighbors_in, features,
           W_in, W_mean, W_std, W_pi, W_ag, W_ad, Wd_mean, Wd_std, Wd_pi,
           _trace=False):
    in_maps = host_prep(nodes1, nodes2, neighbors_out, neighbors_in, features,
                        W_in, W_mean, W_std, W_pi, Wd_mean, Wd_std, Wd_pi)
    nc = _get_nc()
    from concourse.bass_utils import run_bass_kernel_spmd
    res = run_bass_kernel_spmd(nc, in_maps, list(range(NCORES)),
                               trace=_trace)
    if _trace:
        kernel.last_results = res

    out = np.zeros((6, B, D), dtype=np.float32)
    for c in range(NCORES):
        o = res.results[c]["out"]  # [3, D, 64] cols g = s*32+b
        for k in range(3):
            for s in range(2):
                out[s * 3 + k, c * BL:(c + 1) * BL, :] = \
                    o[k][:, s * BL:(s + 1) * BL].T
    return out


# revision 7
# speedup vs baseline: 1.2054x; 1.0010x over previous
"""Trainium2 Bass kernel for the DLSM GNN message-passing model, v2.

Data-parallel over the batch: each of 8 NeuronCores handles 32 nodes of
nodes1 + 32 of nodes2. The sampling indices (fixed RNG columns x input node
ids x input adjacency tables) are computed host-side; the device performs all
feature gathers, neighbor aggregation, and GC/head matmuls.

Key structure per core:
  - Host dedups the ~67k touched feature rows into a <=32768-row fp16 table
    (content-preserving relayout of `features`), so the batched SWDGE
    dma_gather (int16 indices, <=1024 per instruction, 4 queues) can fetch
    all rows in ~67 instructions instead of ~550 serial indirect DMAs.
  - Gather positions are laid out so hop-2 row j of slot (v,h) lands at
    [partition v%128, col (2*(v//128)+h)*10+j] -> neighbor sums are static
    strided DVE reduces; self rows land node-major [v%128, v//128].
  - Compute is fp16 in / f32 PSUM: per 128-node tile, PE-transpose self and
    the two neighbor sums to feature-major, 3-block GC matmul, ACT sigmoid.
    Hop-0 and the three heads reuse the transposed buffers.
"""
import numpy as np
import sys

sys.path.insert(0, '/opt/trn_rl_repo')

import concourse.bass as bass  # noqa: E402
import concourse.tile as tile  # noqa: E402
from concourse import bacc, mybir  # noqa: E402

# ---- problem constants -----------------------------------------------------
N = 200000
F = 128
B = 256
E = 128
D = 64
NCORES = 8
BL = B // NCORES            # 32 base nodes per side per core
NV = 2 * BL * 50            # 3200 hop-1 nodes per core (v-order: s,b,h1,j1)
NT = NV // 128              # 25 tiles
NSLOT = 50                  # hop-2 slots per partition (2 per tile)
N2 = NV * 20                # 64000 hop-2 gather positions
NSB = 3328                  # self(3200) + base(64) + pad(64) positions
GI = 1024                   # max idxs per dma_gather instruction
import os as _os
SINGLE_PACKET = _os.environ.get('K_SP', '0') == '1'
# hop-2 positions are split into 3 segments, each with its own compacted
# fp16 table (unique rows <= draws < 32768 so int16 indices always fit).
SEG = (25600, 25600, 12800)          # hop-2 positions per segment
TROWS = (25600, 25600, 12800 + 3264)  # table rows (seg2 also serves self+base)

F16 = mybir.dt.float16
F32 = mybir.dt.float32
I16 = mybir.dt.int16
SIG = mybir.ActivationFunctionType.Sigmoid
COPY = mybir.ActivationFunctionType.Copy

# Sampling columns fixed by jax.random.key(42) inside the reference.
S1_C1_OUT = [10, 56, 8, 17, 28, 26, 9, 20, 22, 35, 15, 4, 14, 21, 6, 53, 27,
             47, 49, 46, 41, 13, 63, 38, 54]
S1_C1_IN = [19, 59, 37, 12, 34, 31, 29, 1, 3, 0, 24, 40, 26, 11, 25, 23, 13,
            27, 43, 6, 57, 35, 58, 51, 9]
S1_C2_OUT = [57, 36, 9, 2, 34, 3, 6, 11, 0, 21]
S1_C2_IN = [33, 13, 21, 0, 54, 16, 46, 24, 30, 43]
S2_C1_OUT = [9, 7, 34, 52, 15, 35, 54, 30, 10, 16, 42, 56, 51, 28, 12, 19,
             24, 49, 2, 38, 43, 32, 48, 1, 39]
S2_C1_IN = [53, 47, 39, 57, 37, 27, 4, 20, 36, 31, 60, 38, 12, 43, 3, 21, 25,
            58, 48, 52, 23, 35, 15, 28, 7]
S2_C2_OUT = [41, 25, 9, 57, 45, 62, 42, 37, 31, 63]
S2_C2_IN = [40, 34, 60, 56, 2, 14, 6, 32, 50, 25]


def _gather_chunks(total):
    """Split `total` positions into <=GI chunks, each a multiple of 128."""
    out = []
    pos = 0
    while pos < total:
        n = min(GI, total - pos)
        out.append((pos, n))
        pos += n
    return out


def build_program():
    nc = bacc.Bacc("TRN2", target_bir_lowering=False, debug=False,
                   num_swdge_queues=4)

    tabs = [nc.dram_tensor(f"tab{i}", [TROWS[i], F], F16,
                           kind="ExternalInput") for i in range(3)]
    # all int16 index tensors concatenated: idx2 (4000 cols) + idxsb (208)
    icat_d = nc.dram_tensor("icat", [128, N2 // 16 + NSB // 16], I16,
                            kind="ExternalInput")
    # all weights concatenated: w1(3*128) w0(3*128) wh(9*128) wd(3*64) cols
    WCOLS = 15 * 128 + 3 * D + 128
    wcat_d = nc.dram_tensor("wcat", [128, WCOLS], F16, kind="ExternalInput")
    out_d = nc.dram_tensor("out", [3, D, 2 * BL], F32, kind="ExternalOutput")

    with tile.TileContext(nc) as tc:
        with (
            tc.tile_pool(name="const", bufs=1) as cp,
            tc.tile_pool(name="pers", bufs=1) as bp,
            tc.tile_pool(name="piece", bufs=3) as gp,
            tc.tile_pool(name="fmaj", bufs=4) as fp,
            tc.tile_pool(name="small", bufs=2) as sp_,
            tc.tile_pool(name="ps_t", bufs=3, space="PSUM") as pa,
            tc.tile_pool(name="ps_mm", bufs=2, space="PSUM") as pm,
        ):
            wcat = cp.tile([128, WCOLS], F16)
            nc.sync.dma_start(out=wcat[:], in_=wcat_d[:, :])
            w1 = [wcat[:, q * 128:(q + 1) * 128] for q in range(3)]
            w0 = [wcat[:, (3 + q) * 128:(4 + q) * 128] for q in range(3)]
            wh = [[wcat[:, (6 + k * 3 + q) * 128:(7 + k * 3 + q) * 128]
                   for q in range(3)] for k in range(3)]
            wd = [wcat[:, 15 * 128 + k * D:15 * 128 + (k + 1) * D]
                  for k in range(3)]
            ident = wcat[:, 15 * 128 + 3 * D:]

            icat = cp.tile([128, N2 // 16 + NSB // 16], I16)
            nc.sync.dma_start(out=icat[:], in_=icat_d[:, :])
            idx2 = icat[:, 0:N2 // 16]
            idxsb = icat[:, N2 // 16:]

            # ---- self + base feature gather (node-major [q, v//128, f]) ---
            fs = bp.tile([128, (NSB // 128) * F], F16, tag="fs")
            fs3 = fs[:].rearrange("p (c f) -> p c f", f=F)
            qn = 0
            for pos, n in _gather_chunks(NSB):
                nvalid = min(n, 3264 - pos)
                c0 = pos // 128
                nc.gpsimd.dma_gather(
                    fs3[:, c0:c0 + n // 128, :], tabs[2][:, :],
                    idxsb[:, pos // 16:(pos + n) // 16],
                    n, nvalid, F, queue_num=qn,
                    single_packet=SINGLE_PACKET)
                qn = (qn + 1) % 4

            # persistent buffers
            fselfT = bp.tile([128, NV], F16, tag="fselfT")
            h1T = bp.tile([128, NV], F16, tag="h1T")
            msum = bp.tile([128, NSLOT * F], F16, tag="msum")
            msum3 = msum[:].rearrange("p (s f) -> p s f", f=F)

            # ---- front-loaded: self transposes, base transpose, hop-0 -----
            for t in range(NV // 128):
                ps_s = pa.tile([128, 128], F16, tag="ps_s", space="PSUM")
                nc.tensor.matmul(out=ps_s[:], lhsT=fs3[:, t, :], rhs=ident,
                                 start=True, stop=True, is_transpose=True)
                nc.scalar.activation(out=fselfT[:, t * 128:(t + 1) * 128],
                                     in_=ps_s[:], func=COPY)
            ps_b = pa.tile([128, 128], F16, tag="ps_s", space="PSUM")
            nc.tensor.matmul(out=ps_b[:], lhsT=fs3[:, 25, :], rhs=ident,
                             start=True, stop=True, is_transpose=True)
            fbT = sp_.tile([128, 128], F16, tag="fbT")
            nc.scalar.activation(out=fbT[:], in_=ps_b[:], func=COPY)

            m0 = [sp_.tile([128, 64], F16, tag=f"m0_{h}", name=f"m0_{h}")
                  for h in range(2)]
            mh = [sp_.tile([128, 64], F16, tag=f"mh_{h}", name=f"mh_{h}")
                  for h in range(2)]
            with nc.allow_low_precision("fp16 means, tol 2e-2"):
                for h in range(2):
                    nc.vector.tensor_reduce(
                        out=m0[h][:].rearrange("p (g o) -> p g o", o=1),
                        in_=fselfT[:].rearrange("f (g h j) -> f g h j",
                                                h=2, j=25)[:, :, h, :],
                        axis=mybir.AxisListType.X, op=mybir.AluOpType.add)

            ph0 = pm.tile([128, 2 * BL], F32, tag="ph", space="PSUM")
            nc.tensor.matmul(out=ph0[:], lhsT=w0[0], rhs=fbT[:, 0:2 * BL],
                             start=True, stop=False)
            nc.tensor.matmul(out=ph0[:], lhsT=w0[1], rhs=m0[0][:],
                             start=False, stop=False)
            nc.tensor.matmul(out=ph0[:], lhsT=w0[2], rhs=m0[1][:],
                             start=False, stop=True)
            h0T = sp_.tile([128, 2 * BL], F16, tag="h0T")
            nc.scalar.activation(out=h0T[:], in_=ph0[:], func=SIG)

            # ---- hop-2 pipeline: pieces of 2 tiles (40 cols, 5120 idxs) ---
            pieces = []
            pos = 0
            while pos < N2:
                n = min(5120, N2 - pos)
                pieces.append((pos, n))
                pos += n
            for pos, n in pieces:
                seg = 0 if pos < SEG[0] else (1 if pos < SEG[0] + SEG[1]
                                              else 2)
                ptile = gp.tile([128, (n // 128) * F], F16, tag="piece")
                p3 = ptile[:].rearrange("p (c f) -> p c f", f=F)
                for off, gn in _gather_chunks(n):
                    c0 = off // 128
                    nc.gpsimd.dma_gather(
                        p3[:, c0:c0 + gn // 128, :], tabs[seg][:, :],
                        idx2[:, (pos + off) // 16:(pos + off + gn) // 16],
                        gn, gn, F, queue_num=qn,
                        single_packet=SINGLE_PACKET)
                    qn = (qn + 1) % 4
                # neighbor sums: contiguous-run add tree (10 -> 5 -> 2+1)
                s0 = (pos // 128) // 10
                ns = (n // 128) // 10
                x4 = ptile[:].rearrange("p (s j f) -> p s j f", j=10, f=F)
                T = gp.tile([128, ns * 5 * F], F16, tag="redT",
                            name=f"redT_{pos}")
                t4 = T[:].rearrange("p (s j f) -> p s j f", j=5, f=F)
                U = gp.tile([128, ns * 2 * F], F16, tag="redU",
                            name=f"redU_{pos}")
                u4 = U[:].rearrange("p (s j f) -> p s j f", j=2, f=F)
                with nc.allow_low_precision("fp16 neighbor sums, tol 2e-2"):
                    nc.vector.tensor_add(out=t4, in0=x4[:, :, 0:5, :],
                                         in1=x4[:, :, 5:10, :])
                    nc.vector.tensor_add(out=u4, in0=t4[:, :, 0:2, :],
                                         in1=t4[:, :, 2:4, :])
                    nc.vector.tensor_add(out=u4[:, :, 0, :],
                                         in0=u4[:, :, 0, :],
                                         in1=u4[:, :, 1, :])
                    nc.vector.tensor_add(out=msum3[:, s0:s0 + ns, :],
                                         in0=u4[:, :, 0, :],
                                         in1=t4[:, :, 4, :])

                # GC for the tiles completed by this piece
                for t in range(s0 // 2, s0 // 2 + ns // 2):
                    ps_o = pa.tile([128, 128], F16, tag="ps_s", space="PSUM")
                    ps_i = pa.tile([128, 128], F16, tag="ps_s", space="PSUM")
                    nc.tensor.matmul(out=ps_o[:], lhsT=msum3[:, 2 * t, :],
                                     rhs=ident, start=True, stop=True,
                                     is_transpose=True)
                    nc.tensor.matmul(out=ps_i[:], lhsT=msum3[:, 2 * t + 1, :],
                                     rhs=ident, start=True, stop=True,
                                     is_transpose=True)
                    so = fp.tile([128, 128], F16, tag="so")
                    si = fp.tile([128, 128], F16, tag="si")
                    nc.scalar.activation(out=so[:], in_=ps_o[:], func=COPY)
                    nc.scalar.activation(out=si[:], in_=ps_i[:], func=COPY)

                    ph = pm.tile([128, 128], F32, tag="ph", space="PSUM")
                    nc.tensor.matmul(out=ph[:], lhsT=w1[0],
                                     rhs=fselfT[:, t * 128:(t + 1) * 128],
                                     start=True, stop=False)
                    nc.tensor.matmul(out=ph[:], lhsT=w1[1], rhs=so[:],
                                     start=False, stop=False)
                    nc.tensor.matmul(out=ph[:], lhsT=w1[2], rhs=si[:],
                                     start=False, stop=True)
                    nc.scalar.activation(out=h1T[:, t * 128:(t + 1) * 128],
                                         in_=ph[:], func=SIG)

            # ---- layer-1 means + heads (short tail) -----------------------
            # split: groups 0-59 only need h1T[:, :3000] (tiles 0-23), so
            # they reduce while the last piece is still gathering; the final
            # 4 groups (200 cols) are all that gates on the last tile.
            with nc.allow_low_precision("fp16 means, tol 2e-2"):
                for h in range(2):
                    nc.vector.tensor_reduce(
                        out=mh[h][:, 0:60].rearrange("p (g o) -> p g o", o=1),
                        in_=h1T[:, 0:3000].rearrange(
                            "f (g h j) -> f g h j", h=2, j=25)[:, :, h, :],
                        axis=mybir.AxisListType.X, op=mybir.AluOpType.add)
                for h in range(2):
                    nc.vector.tensor_reduce(
                        out=mh[h][:, 60:64].rearrange("p (g o) -> p g o", o=1),
                        in_=h1T[:, 3000:3200].rearrange(
                            "f (g h j) -> f g h j", h=2, j=25)[:, :, h, :],
                        axis=mybir.AxisListType.X, op=mybir.AluOpType.add)

            oall = sp_.tile([D, 3 * 2 * BL], F32, tag="oall")
            oall3 = oall[:].rearrange("d (k g) -> d k g", g=2 * BL)
            for k in range(3):
                pz = pm.tile([128, 2 * BL], F32, tag="ph", space="PSUM")
                nc.tensor.matmul(out=pz[:], lhsT=wh[k][0], rhs=h0T[:],
                                 start=True, stop=False)
                nc.tensor.matmul(out=pz[:], lhsT=wh[k][1], rhs=mh[0][:],
                                 start=False, stop=False)
                nc.tensor.matmul(out=pz[:], lhsT=wh[k][2], rhs=mh[1][:],
                                 start=False, stop=True)
                zh = fp.tile([128, 2 * BL], F16, tag="zh")
                nc.scalar.activation(out=zh[:], in_=pz[:], func=SIG)
                po = pm.tile([D, 2 * BL], F32, tag="po", space="PSUM")
                nc.tensor.matmul(out=po[:], lhsT=wd[k], rhs=zh[:],
                                 start=True, stop=True)
                nc.vector.tensor_copy(out=oall3[:, k, :], in_=po[:])
            nc.sync.dma_start(
                out=out_d[:, :, :].rearrange("k d g -> d k g"),
                in_=oall3)

    nc.compile()
    return nc


_NC_CACHE = None


def _get_nc():
    global _NC_CACHE
    if _NC_CACHE is None:
        _NC_CACHE = build_program()
    return _NC_CACHE


def _wrap16(ids, n):
    """Position-ordered ids -> [128, n//16] int16 wrapped, tiled 8x."""
    a = np.asarray(ids, dtype=np.int64)
    assert a.shape[0] == n and n % 16 == 0
    w = a.astype(np.int16).reshape(-1, 16).T  # [16, n//16]
    return np.ascontiguousarray(np.tile(w, (8, 1)))


def host_prep(nodes1, nodes2, neighbors_out, neighbors_in, features,
              W_in, W_mean, W_std, W_pi, Wd_mean, Wd_std, Wd_pi):
    nodes1 = np.asarray(nodes1, dtype=np.int64)
    nodes2 = np.asarray(nodes2, dtype=np.int64)
    nbr_out = np.asarray(neighbors_out, dtype=np.int64)
    nbr_in = np.asarray(neighbors_in, dtype=np.int64)
    feats = np.asarray(features, dtype=np.float32)

    def scale(w, f):
        w = np.array(w, dtype=np.float32, copy=True)
        w[F:] *= np.float32(f)
        return w.astype(np.float16)

    w1 = scale(W_in, 0.1)
    w0 = scale(W_in, 0.04)
    whs = [scale(W_mean, 0.04), scale(W_std, 0.04), scale(W_pi, 0.04)]
    wds = [np.asarray(w, dtype=np.float16) for w in (Wd_mean, Wd_std, Wd_pi)]

    c1o = (np.array(S1_C1_OUT), np.array(S2_C1_OUT))
    c1i = (np.array(S1_C1_IN), np.array(S2_C1_IN))
    c2o = (np.array(S1_C2_OUT), np.array(S2_C2_OUT))
    c2i = (np.array(S1_C2_IN), np.array(S2_C2_IN))

    in_maps = []
    for c in range(NCORES):
        # v-order: s, b, h1, j1  (concat(out25, in25) per base node)
        s1_parts = []
        base = np.concatenate([nodes1[c * BL:(c + 1) * BL],
                               nodes2[c * BL:(c + 1) * BL]])
        for s, nodes in ((0, nodes1), (1, nodes2)):
            bs = nodes[c * BL:(c + 1) * BL]
            no = nbr_out[bs][:, c1o[s]]           # [BL, 25]
            ni = nbr_in[bs][:, c1i[s]]            # [BL, 25]
            s1_parts.append(np.concatenate([no, ni], 1).reshape(-1))
        s1 = np.concatenate(s1_parts)             # [3200] v-order
        # hop-2 ids in (v, h2, j2) order
        h2o = np.concatenate([nbr_out[s1_parts[0]][:, c2o[0]],
                              nbr_out[s1_parts[1]][:, c2o[1]]])
        h2i = np.concatenate([nbr_in[s1_parts[0]][:, c2i[0]],
                              nbr_in[s1_parts[1]][:, c2i[1]]])
        ids2 = np.concatenate([h2o, h2i], 1).reshape(-1)  # [NV*20]

        # gather-position layouts
        v = np.arange(NV)
        R2 = np.empty(N2, dtype=np.int64)
        vv = np.repeat(v, 20)
        h2 = np.tile(np.repeat(np.arange(2), 10), NV)
        j2 = np.tile(np.arange(10), 2 * NV)
        pos2 = (vv % 128) + 128 * (((vv // 128) * 2 + h2) * 10 + j2)
        R2[pos2] = ids2
        R3 = np.full(NSB, -1, dtype=np.int64)
        R3[(v % 128) + 128 * (v // 128)] = s1
        R3[3200 + np.arange(2 * BL)] = base

        # 3 position segments, each with its own compacted table
        segs = [R2[:SEG[0]], R2[SEG[0]:SEG[0] + SEG[1]],
                np.concatenate([R2[SEG[0] + SEG[1]:], R3[:3264]])]
        tables = []
        locs = []
        for i, req in enumerate(segs):
            # first-use table order: first occurrences walk the table
            # near-sequentially -> HBM row-buffer friendly gathers
            uniq, first_idx, inv = np.unique(req, return_index=True,
                                             return_inverse=True)
            assert len(uniq) <= TROWS[i]
            order = np.argsort(first_idx)
            rank = np.empty_like(order)
            rank[order] = np.arange(len(order))
            table = np.zeros((TROWS[i], F), dtype=np.float16)
            table[:len(uniq)] = feats[uniq[order]].astype(np.float16)
            tables.append(table)
            locs.append(rank[inv])
        L2 = np.concatenate([locs[0], locs[1], locs[2][:SEG[2]]])
        L3 = np.full(NSB, -1, dtype=np.int64)
        L3[:3264] = locs[2][SEG[2]:]

        icat = np.hstack([_wrap16(L2, N2), _wrap16(L3, NSB)])
        blocks = [w1[q * 128:(q + 1) * 128] for q in range(3)]
        blocks += [w0[q * 128:(q + 1) * 128] for q in range(3)]
        for k in range(3):
            blocks += [whs[k][q * 128:(q + 1) * 128] for q in range(3)]
        blocks += wds
        blocks.append(np.eye(128, dtype=np.float16))
        wcat = np.ascontiguousarray(np.hstack(blocks))

        m = {"icat": icat, "wcat": wcat}
        for i in range(3):
            m[f"tab{i}"] = tables[i]
        in_maps.append(m)
    return in_maps


def kernel(nodes1, nodes2, neighbors_out, neighbors_in, features,
           W_in, W_mean, W_std, W_pi, W_ag, W_ad, Wd_mean, Wd_std, Wd_pi,
           _trace=False):
    in_maps = host_prep(nodes1, nodes2, neighbors_out, neighbors_in, features,
                        W_in, W_mean, W_std, W_pi, Wd_mean, Wd_std, Wd_pi)
    nc = _get_nc()
    from concourse.bass_utils import run_bass_kernel_spmd
    res = run_bass_kernel_spmd(nc, in_maps, list(range(NCORES)),
                               trace=_trace)
    if _trace:
        kernel.last_results = res

    out = np.zeros((6, B, D), dtype=np.float32)
    for c in range(NCORES):
        o = res.results[c]["out"]  # [3, D, 64] cols g = s*32+b
        for k in range(3):
            for s in range(2):
                out[s * 3 + k, c * BL:(c + 1) * BL, :] = \
                    o[k][:, s * BL:(s + 1) * BL].T
    return out
